# revision 1
# baseline (speedup 1.0000x reference)
"""Grouped-Query Attention (B=2, S=2048, D=2048, H=32, KV=8, HD=64) on 8 TRN2
NeuronCores, tensor-parallel over KV-head groups (1 KV head + 4 Q heads per
core), with host-side shard/gather.

Per-core dataflow (activations kept feature-on-partitions so every matmul
contracts over the partition dim with no on-device transposition of x):

  phase 1  QKV projection + RoPE
    xT[d-tile, tok-tile] (DMA) -> psum: qa = wqa.T@xT, qb = wqb.T@xT,
    kv = [ka|kb|v].T@xT;  RoPE on DVE directly from PSUM; V transposed back to
    natural [tok, hd] via PE transpose (PV matmul lhsT needs it).
  phase 2  attention per (batch, q-tile of 512), causal-block-skipped
    scoresT[sk=128, q=512] = krot.T @ qrot (heads packed 2-per-PE-pass via row
    groups);  probsT = exp(scale*scoresT) (ACT, no max-subtraction: |s|<=6
    verified on the actual distribution);  diagonal blocks masked by a 0/1
    mask multiply;  PV accumulates outT[65, 512] = [1|V].T @ probsT over
    sk-tiles (row 0 = softmax denominator via the ones column).
  phase 3  output projection y[tok, 512] = attn_outT.T @ wo, DMA out.

Host sums the 8 per-core partial y (wo is row-sharded).
"""

import contextlib
import os
import numpy as np
import jax.numpy as jnp

import concourse.bass as bass
import concourse.tile as tile
from concourse import bacc, mybir
from concourse.bass_utils import run_bass_kernel_spmd
from concourse.masks import make_identity

B, S, D = 2, 2048, 2048
H, KV, HD = 32, 8, 64
T = B * S
NCORES = 8
HPC = H // NCORES          # 4 query heads per core
SCALE = 1.0 / np.sqrt(HD)
THETA = 10000.0
NQT = T // 512             # 8 token tiles of 512
REPLICATED = {"xT", "cos4", "sin4", "mask"}  # same bytes on every core
NDT = D // 128             # 16 contraction tiles
F32 = mybir.dt.float32

# fp32r: 4-byte fp32 storage, reduced-precision full-rate matmul (1 cyc/row at
# free-dim >= 256 vs 4 for strict fp32).  Flip to "0" to fall back.
USE_F32R = os.environ.get("GQA_F32R", "1") == "1"
MM_DT = mybir.dt.float32r if USE_F32R else mybir.dt.float32


def _bc(ap):
    # DRAM-side view for DMA into an MM_DT tile (bit-identical 4-byte cast)
    return ap.bitcast(MM_DT) if USE_F32R else ap


def _build_program():
    nc = bacc.Bacc("TRN2", target_bir_lowering=False, debug=False)

    xT = nc.dram_tensor("xT", [D, T], F32, kind="ExternalInput")
    wq = nc.dram_tensor("wq", [D, 2 * HPC * 32], F32, kind="ExternalInput")
    wkv = nc.dram_tensor("wkv", [D, 128], F32, kind="ExternalInput")
    wo = nc.dram_tensor("wo", [HPC * HD, D], F32, kind="ExternalInput")
    cos4 = nc.dram_tensor("cos4", [128, S], F32, kind="ExternalInput")
    sin4 = nc.dram_tensor("sin4", [128, S], F32, kind="ExternalInput")
    maskd = nc.dram_tensor("mask", [128, 896], F32, kind="ExternalInput")
    y = nc.dram_tensor("y", [T, D], F32, kind="ExternalOutput")

    with tile.TileContext(nc) as tc:
        _body(tc, nc, xT, wq, wkv, wo, cos4, sin4, maskd, y)
    nc.compile()
    return nc


def _body(tc, nc, xT, wq, wkv, wo, cos4, sin4, maskd, y):
    TT = mybir.AluOpType
    SC_NAMES = ["ps_a", "ps_b", "ps_c", "ps_t"]
    ctx = contextlib.ExitStack()
    with ctx:
        const = ctx.enter_context(tc.tile_pool(name="const", bufs=1))
        persist = ctx.enter_context(tc.tile_pool(name="persist", bufs=1))
        xs = ctx.enter_context(tc.tile_pool(name="xs", bufs=3))
        rtmp = ctx.enter_context(tc.tile_pool(name="rtmp", bufs=1))
        probs = ctx.enter_context(tc.tile_pool(name="probs", bufs=1))
        norm = ctx.enter_context(tc.tile_pool(name="norm", bufs=1))
        yout = ctx.enter_context(tc.tile_pool(name="yout", bufs=2))
        # PSUM is 8 banks of [128 x 512 f32].  Tags: ps_a/b/c/t shared across
        # phases (proj accumulators -> score tiles -> out-proj), pv0-3 are the
        # PV accumulators.  Total static reservation = exactly 8 banks.
        psum = ctx.enter_context(tc.tile_pool(name="psum", bufs=1, space="PSUM"))

        # ---- constants ----
        wq_sb = const.tile([128, NDT, 256], MM_DT, name="wq_sb")
        nc.sync.dma_start(out=wq_sb, in_=_bc(wq[:, :].rearrange("(t p) c -> p t c", p=128)))
        wkv_sb = const.tile([128, NDT, 128], MM_DT, name="wkv_sb")
        nc.sync.dma_start(out=wkv_sb, in_=_bc(wkv[:, :].rearrange("(t p) c -> p t c", p=128)))
        wo_sb = const.tile([128, 2, D], MM_DT, name="wo_sb")
        nc.sync.dma_start(out=wo_sb, in_=_bc(wo[:, :].rearrange("(t p) c -> p t c", p=128)))
        cos_sb = const.tile([128, S], F32, name="cos_sb")
        nc.sync.dma_start(out=cos_sb, in_=cos4[:, :])
        sin_sb = const.tile([128, S], F32, name="sin_sb")
        nc.sync.dma_start(out=sin_sb, in_=sin4[:, :])
        mask_sb = const.tile([128, 896], MM_DT, name="mask_sb")
        nc.sync.dma_start(out=mask_sb, in_=_bc(maskd[:, :]))
        ident = const.tile([64, 64], F32, name="ident")
        make_identity(nc, ident)

        # ---- persistent activations ----
        qrot = persist.tile([128, 2, T], MM_DT, name="qrot")   # [rowgrp, headpair, tok]
        krot2 = persist.tile([128, T], MM_DT, name="krot2")    # rows 64:128 = copy of 0:64
        vnat = persist.tile([128, T // 128, 65], MM_DT, name="vnat")  # [tok%128, toktile, hd+1]
        a0 = persist.tile([128, T], MM_DT, name="a0")          # attn outT, heads 0,1
        a1 = persist.tile([128, T], MM_DT, name="a1")          # attn outT, heads 2,3
        ones_c = const.tile([128, T // 128, 1], F32, name="ones_c")
        nc.vector.memset(ones_c, 1.0)
        nc.vector.tensor_copy(out=vnat[:, :, 64:65], in_=ones_c)

        # ================= phase 1: projections + rope =================
        for qt in range(NQT):
            pos0 = (qt % 4) * 512
            tok0 = qt * 512
            qa_ps = psum.tile([128, 512], F32, name="ps_a")
            qb_ps = psum.tile([128, 512], F32, name="ps_b")
            kv_ps = psum.tile([128, 512], F32, name="ps_c")
            for d in range(NDT):
                xt = xs.tile([128, 512], MM_DT, name="xt")
                nc.sync.dma_start(out=xt, in_=_bc(xT[d * 128:(d + 1) * 128, tok0:tok0 + 512]))
                st, sp = d == 0, d == NDT - 1
                nc.tensor.matmul(out=qa_ps, lhsT=(wq_sb[:, d, 0:128]), rhs=(xt),
                                 start=st, stop=sp)
                nc.tensor.matmul(out=qb_ps, lhsT=(wq_sb[:, d, 128:256]), rhs=(xt),
                                 start=st, stop=sp)
                nc.tensor.matmul(out=kv_ps, lhsT=(wkv_sb[:, d, :]), rhs=(xt),
                                 start=st, stop=sp)
            cs = cos_sb[:, pos0:pos0 + 512]
            sn = sin_sb[:, pos0:pos0 + 512]
            # Q rope on [128, 512] (row 32h+r = head h dim r); both reads of
            # each psum issued back-to-back so the bank frees early.
            t_x = rtmp.tile([128, 512], F32, name="t_x")
            t_x2 = rtmp.tile([128, 512], F32, name="t_x2")
            nc.vector.tensor_tensor(out=t_x, in0=qa_ps, in1=cs, op=TT.mult)
            nc.vector.tensor_tensor(out=t_x2, in0=qa_ps, in1=sn, op=TT.mult)
            t_y = rtmp.tile([128, 512], F32, name="t_y")
            t_y2 = rtmp.tile([128, 512], F32, name="t_y2")
            nc.vector.tensor_tensor(out=t_y, in0=qb_ps, in1=sn, op=TT.mult)
            nc.vector.tensor_tensor(out=t_y2, in0=qb_ps, in1=cs, op=TT.mult)
            qra = rtmp.tile([128, 512], MM_DT, name="qra")
            qrb = rtmp.tile([128, 512], MM_DT, name="qrb")
            nc.vector.tensor_tensor(out=qra, in0=t_x, in1=t_y, op=TT.subtract)
            nc.vector.tensor_tensor(out=qrb, in0=t_x2, in1=t_y2, op=TT.add)
            # remap into [rowgrp(h%2), headpair(h//2)] layout for packed scores
            for h in range(HPC):
                rb = (h % 2) * 64
                blk = h // 2
                nc.sync.dma_start(out=qrot[rb:rb + 32, blk, tok0:tok0 + 512],
                                  in_=qra[32 * h:32 * h + 32, :])
                nc.sync.dma_start(out=qrot[rb + 32:rb + 64, blk, tok0:tok0 + 512],
                                  in_=qrb[32 * h:32 * h + 32, :])
            # K rope (single kv head): rows 0:32 ka, 32:64 kb of kv_ps; V copy.
            k_x = rtmp.tile([32, 512], F32, name="k_x")
            k_x2 = rtmp.tile([32, 512], F32, name="k_x2")
            k_y = rtmp.tile([32, 512], F32, name="k_y")
            k_y2 = rtmp.tile([32, 512], F32, name="k_y2")
            vt = rtmp.tile([64, 512], F32, name="vt")
            nc.vector.tensor_tensor(out=k_x, in0=kv_ps[0:32], in1=cs[0:32], op=TT.mult)
            nc.vector.tensor_tensor(out=k_x2, in0=kv_ps[0:32], in1=sn[0:32], op=TT.mult)
            nc.vector.tensor_tensor(out=k_y, in0=kv_ps[32:64], in1=sn[0:32], op=TT.mult)
            nc.vector.tensor_tensor(out=k_y2, in0=kv_ps[32:64], in1=cs[0:32], op=TT.mult)
            nc.vector.tensor_copy(out=vt, in_=kv_ps[64:128])
            nc.vector.tensor_tensor(out=krot2[0:32, tok0:tok0 + 512], in0=k_x,
                                    in1=k_y, op=TT.subtract)
            nc.vector.tensor_tensor(out=krot2[32:64, tok0:tok0 + 512], in0=k_x2,
                                    in1=k_y2, op=TT.add)
            # V back to natural layout [tok, hd] via PE transpose
            for k4 in range(4):
                tp = psum.tile([128, 64], F32, name="ps_t")
                nc.tensor.transpose(tp, vt[:, k4 * 128:(k4 + 1) * 128], ident)
                nc.vector.tensor_copy(out=vnat[:, qt * 4 + k4, 0:64], in_=tp)

        # replicate krot rows 0:64 -> 64:128 so head pairs pack into row groups
        nc.sync.dma_start(out=krot2[64:128, :], in_=krot2[0:64, :])

        # ================= phase 2: attention =================
        for b in range(B):
            for jq in range(4):
                tq = b * S + jq * 512
                pv = [psum.tile([65, 512], F32, name=f"ps_pv{h}") for h in range(HPC)]
                ni = 4 * jq + 4
                for i in range(ni):
                    tk = b * S + i * 128
                    sc = [psum.tile([128, 512], F32, name=SC_NAMES[h])
                          for h in range(HPC)]
                    for h in range(HPC):
                        rb = (h % 2) * 64
                        blk = h // 2
                        nc.tensor.matmul(
                            out=sc[h],
                            lhsT=(krot2[rb:rb + 64, tk:tk + 128]),
                            rhs=(qrot[rb:rb + 64, blk, tq:tq + 512]),
                            start=True, stop=True)
                    for h in range(HPC):
                        pt = probs.tile([128, 512], MM_DT, name=f"pt{h}")
                        nc.scalar.activation(out=pt, in_=sc[h],
                                             func=mybir.ActivationFunctionType.Exp,
                                             scale=float(SCALE))
                        if i >= 4 * jq:  # diagonal block: causal mask
                            roff = 128 * i - 512 * jq
                            nc.vector.tensor_tensor(
                                out=pt, in0=pt,
                                in1=mask_sb[:, 384 - roff:896 - roff], op=TT.mult)
                        nc.tensor.matmul(out=pv[h], lhsT=(vnat[:, b * 16 + i, :]),
                                         rhs=(pt), start=(i == 0), stop=(i == ni - 1))
                # normalize: row 64 of pv[h] is the softmax denominator
                sums = norm.tile([1, HPC * 512], F32, name="sums")
                for h in range(HPC):
                    nc.scalar.copy(out=sums[0:1, h * 512:(h + 1) * 512],
                                   in_=pv[h][64:65])
                rec = norm.tile([1, HPC * 512], F32, name="rec")
                nc.vector.reciprocal(out=rec, in_=sums)
                for h in range(HPC):
                    rbc = norm.tile([64, 512], F32, name="rbc")
                    nc.gpsimd.partition_broadcast(rbc, rec[0:1, h * 512:(h + 1) * 512])
                    dst = a0 if h < 2 else a1
                    rb = (h % 2) * 64
                    nc.vector.tensor_tensor(out=dst[rb:rb + 64, tq:tq + 512],
                                            in0=pv[h][0:64], in1=rbc, op=TT.mult)

        # ================= phase 3: output projection =================
        for tt in range(T // 128):
            for n in range(D // 512):
                yo = psum.tile([128, 512], F32, name=SC_NAMES[(tt * 4 + n) % 4])
                nc.tensor.matmul(out=yo, lhsT=(a0[:, tt * 128:(tt + 1) * 128]),
                                 rhs=(wo_sb[:, 0, n * 512:(n + 1) * 512]),
                                 start=True, stop=False)
                nc.tensor.matmul(out=yo, lhsT=(a1[:, tt * 128:(tt + 1) * 128]),
                                 rhs=(wo_sb[:, 1, n * 512:(n + 1) * 512]),
                                 start=False, stop=True)
                ys = yout.tile([128, 512], F32, name="ys")
                if n % 2 == 0:
                    nc.scalar.copy(out=ys, in_=yo)
                else:
                    nc.vector.tensor_copy(out=ys, in_=yo)
                nc.sync.dma_start(out=y[tt * 128:(tt + 1) * 128, n * 512:(n + 1) * 512],
                                  in_=ys)


_CACHE = {}


def _get_program():
    if "nc" not in _CACHE:
        _CACHE["nc"] = _build_program()
    return _CACHE["nc"]


def _get_runner():
    """Cached jitted shard_map executable over 8 cores (avoids per-call
    retrace that run_bass_kernel_spmd pays)."""
    if "runner" in _CACHE:
        return _CACHE["runner"]
    import jax
    from jax.sharding import Mesh, PartitionSpec
    from jax.experimental.shard_map import shard_map
    from concourse import bass2jax
    from concourse.bass2jax import _bass_exec_p

    bass2jax.install_neuronx_cc_hook()
    nc = _get_program()
    partition_name = nc.partition_id_tensor.name if nc.partition_id_tensor else None
    in_names, out_names, out_avals = [], [], []
    for alloc in nc.m.functions[0].allocations:
        if not isinstance(alloc, mybir.MemoryLocationSet):
            continue
        name = alloc.memorylocations[0].name
        if alloc.kind == "ExternalInput":
            if name != partition_name:
                in_names.append(name)
        elif alloc.kind == "ExternalOutput":
            out_names.append(name)
            out_avals.append(jax.core.ShapedArray(
                tuple(alloc.tensor_shape), mybir.dt.np(alloc.dtype)))
    n_params = len(in_names)
    n_outs = len(out_avals)
    all_in = list(in_names) + list(out_names)
    if partition_name is not None:
        all_in.append(partition_name)

    def _body(*args):
        operands = list(args)
        if partition_name is not None:
            operands.append(bass2jax.partition_id_tensor())
        return tuple(_bass_exec_p.bind(
            *operands,
            out_avals=tuple(out_avals),
            in_names=tuple(all_in),
            out_names=tuple(out_names),
            lowering_input_output_aliases=(),
            sim_require_finite=True,
            sim_require_nnan=True,
            nc=nc,
        ))

    devices = jax.devices()[:NCORES]
    mesh = Mesh(np.asarray(devices), ("core",))
    # xT / rope tables / mask are identical on every core: feed them
    # replicated (P()) so the host uploads one copy + on-device all-gather,
    # instead of 8 copies through the tunnel.
    in_specs = tuple(
        PartitionSpec() if n in REPLICATED else PartitionSpec("core")
        for n in in_names) + (PartitionSpec("core"),) * n_outs
    sharded = jax.jit(
        shard_map(_body, mesh=mesh,
                  in_specs=in_specs,
                  out_specs=(PartitionSpec("core"),) * n_outs,
                  check_rep=False),
        donate_argnums=tuple(range(n_params, n_params + n_outs)),
        keep_unused=True)

    from jax.sharding import NamedSharding
    rep = NamedSharding(mesh, PartitionSpec())
    shd = NamedSharding(mesh, PartitionSpec("core"))
    gather = jax.jit(lambda a: a, out_shardings=rep)   # upload-shard -> all-gather
    zeros = jax.jit(lambda: jnp.zeros((NCORES * T, D), jnp.float32),
                    out_shardings=shd)
    reduce_y = jax.jit(lambda yc: yc.reshape(NCORES, T, D)
                       .sum(0, dtype=jnp.float32), out_shardings=rep)
    _CACHE["runner"] = (sharded, in_names, out_names, out_avals,
                        mesh, rep, shd, gather, zeros, reduce_y)
    return _CACHE["runner"]


def _host_inputs(x, wq, wk, wv, wo):
    x = np.asarray(x, np.float32)
    wq = np.asarray(wq, np.float32)
    wk = np.asarray(wk, np.float32)
    wv = np.asarray(wv, np.float32)
    wo = np.asarray(wo, np.float32)

    xT = np.ascontiguousarray(x.reshape(T, D).T)

    inv = 1.0 / (THETA ** (np.arange(0, HD, 2, dtype=np.float64) / HD))
    fr = np.outer(inv, np.arange(S, dtype=np.float64))   # [32, S]
    cosT = np.cos(fr).astype(np.float32)
    sinT = np.sin(fr).astype(np.float32)
    cos4 = np.ascontiguousarray(np.tile(cosT, (4, 1)))
    sin4 = np.ascontiguousarray(np.tile(sinT, (4, 1)))

    u = np.arange(896)[None, :]
    p = np.arange(128)[:, None]
    mask = (u >= p + 384).astype(np.float32)

    in_maps = []
    for c in range(NCORES):
        cols_a, cols_b = [], []
        for h in range(HPC):
            base = (HPC * c + h) * HD
            cols_a.append(wq[:, base:base + 32])
            cols_b.append(wq[:, base + 32:base + 64])
        wq_c = np.ascontiguousarray(np.concatenate(cols_a + cols_b, axis=1))
        kb = c * HD
        wkv_c = np.ascontiguousarray(np.concatenate(
            [wk[:, kb:kb + 32], wk[:, kb + 32:kb + 64], wv[:, kb:kb + HD]], axis=1))
        wo_c = np.ascontiguousarray(wo[c * HPC * HD:(c + 1) * HPC * HD, :])
        in_maps.append({"xT": xT, "wq": wq_c, "wkv": wkv_c, "wo": wo_c,
                        "cos4": cos4, "sin4": sin4, "mask": mask})
    return in_maps


def _stage_inputs(in_maps):
    """Upload inputs: replicated tensors go up as 1/8 shards and are
    all-gathered on device; per-core tensors upload as the usual concat."""
    import jax
    (sharded, in_names, out_names, out_avals,
     mesh, rep, shd, gather, zeros, reduce_y) = _get_runner()
    staged = []
    for n in in_names:
        if n in REPLICATED:
            a = in_maps[0][n]
            if a.shape[0] % NCORES == 0:
                staged.append(gather(jax.device_put(a, shd)))
            else:
                staged.append(jax.device_put(a, rep))
        else:
            cat = np.concatenate([m[n] for m in in_maps], axis=0)
            staged.append(jax.device_put(cat, shd))
    return staged


def kernel(x, wq, wk, wv, wo):
    import jax
    (sharded, in_names, out_names, out_avals,
     mesh, rep, shd, gather, zeros, reduce_y) = _get_runner()
    in_maps = _host_inputs(x, wq, wk, wv, wo)
    staged = _stage_inputs(in_maps)
    out_arrs = sharded(*staged, zeros())
    ysum = reduce_y(out_arrs[out_names.index("y")])
    return np.asarray(ysum).reshape(B, S, D)



# revision 25
# speedup vs baseline: 112.6315x; 112.6315x over previous
"""Grouped-Query Attention (B=2, S=2048, D=2048, H=32, KV=8, HD=64) on 8 TRN2
NeuronCores, tensor-parallel over KV-head groups (1 KV head + 4 Q heads per
core), with host-side shard/gather.

v2: bf16 compute (halved HBM traffic, 2x DVE on 16-bit), pair-packed score
tiles with one exp per head-pair (Activation engine is the phase-2
bottleneck), PSUM retagged so scores double-buffer and the output projection
of tile jq overlaps attention of jq+1, engine rebalance (rope-K + copies on
Pool/Act, masks + normalize on DVE).

Per-core dataflow (activations feature-on-partitions; every matmul contracts
over the partition dim, no transposition of x):

  phase 1  QKV projection + RoPE, per 512-token tile
    psum: qa/qb (q ra/rb halves of 4 heads), kv = [ka|kb|v]; RoPE combines
    write q directly into qrot[128, hh, T] (head pairs in partition halves),
    k into krot rows 0:64 (replicated to 64:128 by per-tile DMA); V back to
    natural [tok, hd] via PE transpose.
  phase 2  attention per (batch, q-tile of 512), causal-block-skipped,
    head pairs processed in sequential i-loops:
      scoresT[2 x 128 kpos, 512 q] -> one exp per pair -> diag mask ->
      PV accumulate [65, 512] per head (row 64 = softmax denominator via
      ones column of V);  normalize -> a0/a1 (bf16);
    then the 512-token output projection y = [a0;a1].T @ wo on its own psum
    tags, overlapping the next q-tile's attention; y written bf16, host
    reduces the 8 row-sharded partials in f32.
"""

import contextlib
import numpy as np
import jax.numpy as jnp

import concourse.bass as bass
import concourse.tile as tile
from concourse import bacc, mybir
from concourse.masks import make_identity

B, S, D = 2, 2048, 2048
H, KV, HD = 32, 8, 64
T = B * S
NCORES = 8
HPC = H // NCORES          # 4 query heads per core
SCALE = 1.0 / np.sqrt(HD)
THETA = 10000.0
NQT = T // 512             # 8 token tiles of 512
REPLICATED = {"xT", "cos4", "sin4", "mask"}  # same bytes on every core
NDT = D // 128             # 16 contraction tiles
F32 = mybir.dt.float32
BF16 = mybir.dt.bfloat16
NPBF16 = mybir.dt.np(BF16)


def _build_program():
    nc = bacc.Bacc("TRN2", target_bir_lowering=False, debug=False)

    xT = nc.dram_tensor("xT", [D, T], BF16, kind="ExternalInput")
    wq = nc.dram_tensor("wq", [D, 2 * HPC * 32], BF16, kind="ExternalInput")
    wkv = nc.dram_tensor("wkv", [D, 128], BF16, kind="ExternalInput")
    wo = nc.dram_tensor("wo", [HPC * HD, D], BF16, kind="ExternalInput")
    cos4 = nc.dram_tensor("cos4", [128, S], BF16, kind="ExternalInput")
    sin4 = nc.dram_tensor("sin4", [128, S], BF16, kind="ExternalInput")
    maskd = nc.dram_tensor("mask", [128, 896], BF16, kind="ExternalInput")
    y = nc.dram_tensor("y", [T, D], BF16, kind="ExternalOutput")

    with tile.TileContext(nc) as tc:
        _body(tc, nc, xT, wq, wkv, wo, cos4, sin4, maskd, y)
    nc.compile()
    return nc


def _body(tc, nc, xT, wq, wkv, wo, cos4, sin4, maskd, y):
    TT = mybir.AluOpType
    EXP = mybir.ActivationFunctionType.Exp
    ctx = contextlib.ExitStack()
    with ctx:
        const = ctx.enter_context(tc.tile_pool(name="const", bufs=1))
        persist = ctx.enter_context(tc.tile_pool(name="persist", bufs=1))
        xs = ctx.enter_context(tc.tile_pool(name="xs", bufs=18))
        rtmp = ctx.enter_context(tc.tile_pool(name="rtmp", bufs=1))
        probs = ctx.enter_context(tc.tile_pool(name="probs", bufs=4))
        yout = ctx.enter_context(tc.tile_pool(name="yout", bufs=3))
        norm = ctx.enter_context(tc.tile_pool(name="norm", bufs=2))

        # PSUM: 16KB/partition, allocated exactly:
        #   T0..T3: [128,512] f32 (2KB = 1 bank each)
        #   S0,S1 : [128,2,512] f32 (4KB = 2 banks each)
        # phase 1: qa/qb even qt -> T0/T1, odd qt -> S0 halves; kv=T2, tp=T3
        # phase 2: scores alternate S0/S1; pv pair -> T2/T3; out-proj T0/T1
        psum = ctx.enter_context(tc.tile_pool(name="psum", bufs=1, space="PSUM"))

        def pT(i):
            return psum.tile([128, 512], F32, name=f"psT{i}")

        def pS(i):
            return psum.tile([128, 2, 512], F32, name=f"psS{i}")

        # ---- constants ----
        wq_sb = const.tile([128, NDT, 256], BF16, name="wq_sb")
        nc.sync.dma_start(out=wq_sb, in_=wq[:, :].rearrange("(t p) c -> p t c", p=128))
        wkv_sb = const.tile([128, NDT, 128], BF16, name="wkv_sb")
        nc.sync.dma_start(out=wkv_sb, in_=wkv[:, :].rearrange("(t p) c -> p t c", p=128))
        wo_sb = const.tile([128, 2, D], BF16, name="wo_sb")
        nc.sync.dma_start(out=wo_sb, in_=wo[:, :].rearrange("(t p) c -> p t c", p=128))
        cos_sb = const.tile([128, S], BF16, name="cos_sb")
        nc.sync.dma_start(out=cos_sb, in_=cos4[:, :])
        sin_sb = const.tile([128, S], BF16, name="sin_sb")
        nc.sync.dma_start(out=sin_sb, in_=sin4[:, :])
        mask_sb = const.tile([128, 896], BF16, name="mask_sb")
        nc.sync.dma_start(out=mask_sb, in_=maskd[:, :])
        ident = const.tile([64, 64], F32, name="ident")
        make_identity(nc, ident)

        # ---- persistent activations ----
        # qrot: [64*pair + (0:32 ra | 32:64 rb), head-in-pair, tok]
        qrot = persist.tile([128, 2, T], BF16, name="qrot")
        krot = persist.tile([128, T], BF16, name="krot")   # 64:128 = replica
        vnat = persist.tile([128, T // 128, 65], BF16, name="vnat")
        a0 = persist.tile([128, T], BF16, name="a0")       # heads 0,1
        a1 = persist.tile([128, T], BF16, name="a1")       # heads 2,3
        ones_c = const.tile([128, T // 128, 1], F32, name="ones_c")
        nc.vector.memset(ones_c, 1.0)
        nc.vector.tensor_copy(out=vnat[:, :, 64:65], in_=ones_c)

        # ================= phase 1: projections + rope =================
        for qt in range(NQT):
            pos0 = (qt % 4) * 512
            tok0 = qt * 512
            if qt % 2 == 0:
                qa_ps, qb_ps = pT(0), pT(1)
            else:
                s_ps = pS(0)
                qa_ps, qb_ps = s_ps[:, 0, :], s_ps[:, 1, :]
            kv_ps = pT(2)
            # kv projection first: the single-buffered kv bank is consumed by
            # rope-K (DVE) while PE continues with the qa/qb matmuls below.
            xts = []
            for d in range(NDT):
                xt = xs.tile([128, 512], BF16, name="xt")
                nc.sync.dma_start(out=xt, in_=xT[d * 128:(d + 1) * 128, tok0:tok0 + 512])
                xts.append(xt)
                nc.tensor.matmul(out=kv_ps, lhsT=wkv_sb[:, d, :], rhs=xt,
                                 start=d == 0, stop=d == NDT - 1)
            cs = cos_sb[:, pos0:pos0 + 512]
            sn = sin_sb[:, pos0:pos0 + 512]
            # stage k through SBUF bf16 (Act copy, idle engine) so every rope
            # TT runs in the DVE 16-bit fast mode
            kk = rtmp.tile([64, 512], BF16, name="kk")
            nc.scalar.copy(out=kk, in_=kv_ps[0:64])
            k_x = rtmp.tile([32, 512], BF16, name="k_x")
            k_x2 = rtmp.tile([32, 512], BF16, name="k_x2")
            k_y = rtmp.tile([32, 512], BF16, name="k_y")
            k_y2 = rtmp.tile([32, 512], BF16, name="k_y2")
            nc.vector.tensor_tensor(out=k_x, in0=kk[0:32], in1=cs[0:32], op=TT.mult)
            nc.vector.tensor_tensor(out=k_x2, in0=kk[0:32], in1=sn[0:32], op=TT.mult)
            # cos/sin tables are 4x-tiled over partitions, so rows 32:64
            # equal rows 0:32; index them to satisfy the equal-base-partition
            # rule for SBUF-SBUF TensorTensor.
            nc.vector.tensor_tensor(out=k_y, in0=kk[32:64], in1=sn[32:64], op=TT.mult)
            nc.vector.tensor_tensor(out=k_y2, in0=kk[32:64], in1=cs[32:64], op=TT.mult)
            vt = rtmp.tile([64, 512], F32, name="vt")
            nc.scalar.copy(out=vt, in_=kv_ps[64:128])
            nc.vector.tensor_tensor(out=krot[0:32, tok0:tok0 + 512], in0=k_x,
                                    in1=k_y, op=TT.subtract)
            nc.vector.tensor_tensor(out=krot[32:64, tok0:tok0 + 512], in0=k_x2,
                                    in1=k_y2, op=TT.add)
            nc.sync.dma_start(out=krot[64:128, tok0:tok0 + 512],
                              in_=krot[0:64, tok0:tok0 + 512])
            for d in range(NDT):
                st, sp = d == 0, d == NDT - 1
                nc.tensor.matmul(out=qa_ps, lhsT=wq_sb[:, d, 0:128], rhs=xts[d],
                                 start=st, stop=sp)
                nc.tensor.matmul(out=qb_ps, lhsT=wq_sb[:, d, 128:256], rhs=xts[d],
                                 start=st, stop=sp)
            # V -> natural [tok, hd] via PE transpose into T3, Pool copy out
            tp = pT(3)
            for k4 in range(4):
                nc.tensor.transpose(tp[:, 64 * k4:64 * k4 + 64],
                                    vt[:, 128 * k4:128 * k4 + 128], ident)
            for k4 in range(4):
                nc.scalar.copy(out=vnat[:, qt * 4 + k4, 0:64],
                               in_=tp[:, 64 * k4:64 * k4 + 64])
            # Q rope on DVE: [128, 512] (row 32h+r = head h, ra/rb dim r);
            # staged to SBUF bf16 by Act copies for the DVE fast mode
            qa_sb = rtmp.tile([128, 512], BF16, name="qa_sb")
            qb_sb = rtmp.tile([128, 512], BF16, name="qb_sb")
            nc.scalar.copy(out=qa_sb, in_=qa_ps)
            nc.scalar.copy(out=qb_sb, in_=qb_ps)
            t_x = rtmp.tile([128, 512], BF16, name="t_x")
            t_x2 = rtmp.tile([128, 512], BF16, name="t_x2")
            t_y = rtmp.tile([128, 512], BF16, name="t_y")
            t_y2 = rtmp.tile([128, 512], BF16, name="t_y2")
            nc.vector.tensor_tensor(out=t_x, in0=qa_sb, in1=cs, op=TT.mult)
            nc.vector.tensor_tensor(out=t_x2, in0=qa_sb, in1=sn, op=TT.mult)
            nc.vector.tensor_tensor(out=t_y, in0=qb_sb, in1=sn, op=TT.mult)
            nc.vector.tensor_tensor(out=t_y2, in0=qb_sb, in1=cs, op=TT.mult)
            for h in range(HPC):
                p, hh = h // 2, h % 2
                r0 = 32 * h
                nc.vector.tensor_tensor(
                    out=qrot[64 * p:64 * p + 32, hh, tok0:tok0 + 512],
                    in0=t_x[r0:r0 + 32], in1=t_y[r0:r0 + 32], op=TT.subtract)
                nc.vector.tensor_tensor(
                    out=qrot[64 * p + 32:64 * p + 64, hh, tok0:tok0 + 512],
                    in0=t_x2[r0:r0 + 32], in1=t_y2[r0:r0 + 32], op=TT.add)

        # ================= phase 2: attention + out-proj =================
        # Out-projection of q-tile jq runs on its own psum tags (T0/T1),
        # drained one 2-matmul unit per attention iteration of the NEXT
        # q-tile so the in-order PE queue never stalls the exp stream.
        def emit_outproj_unit(ts, n, par):
            yo = pT(par)
            nc.tensor.matmul(out=yo, lhsT=a0[:, ts:ts + 128],
                             rhs=wo_sb[:, 0, 512 * n:512 * n + 512],
                             start=True, stop=False)
            nc.tensor.matmul(out=yo, lhsT=a1[:, ts:ts + 128],
                             rhs=wo_sb[:, 1, 512 * n:512 * n + 512],
                             start=False, stop=True)
            ys = yout.tile([128, 512], BF16, name="ys")
            nc.vector.tensor_copy(out=ys, in_=yo)
            nc.sync.dma_start(out=y[ts:ts + 128, 512 * n:512 * n + 512], in_=ys)

        pending = []
        for b in range(B):
            for jq in range(4):
                tq = b * S + jq * 512
                ni = 4 * jq + 4
                for p in range(2):  # head pair: heads 2p, 2p+1
                    pv = [pT(2), pT(3)]
                    # 2-ahead software pipeline: PE queue per i is
                    # [scores(i), pv(i-2), outproj-unit] so exp(i-1) is never
                    # behind a matmul that waits on it.
                    pts = {}
                    for i in range(ni):
                        tk = b * S + i * 128
                        sc = pS(i % 2)
                        for hh in range(2):
                            nc.tensor.matmul(
                                out=sc[:, hh, :],
                                lhsT=krot[64 * p:64 * p + 64, tk:tk + 128],
                                rhs=qrot[64 * p:64 * p + 64, hh, tq:tq + 512],
                                start=True, stop=True)
                        pt = probs.tile([128, 2, 512], BF16, name="pt")
                        nc.scalar.activation(out=pt, in_=sc, func=EXP,
                                             scale=float(SCALE))
                        if i >= 4 * jq:  # diagonal block: causal mask
                            roff = 128 * i - 512 * jq
                            for hh in range(2):
                                nc.vector.tensor_tensor(
                                    out=pt[:, hh, :], in0=pt[:, hh, :],
                                    in1=mask_sb[:, 384 - roff:896 - roff],
                                    op=TT.mult)
                        pts[i] = pt
                        if i >= 2:
                            j = i - 2
                            ptj = pts.pop(j)
                            for hh in range(2):
                                nc.tensor.matmul(out=pv[hh][0:65, :],
                                                 lhsT=vnat[:, b * 16 + j, :],
                                                 rhs=ptj[:, hh, :],
                                                 start=(j == 0), stop=False)
                        if pending:
                            emit_outproj_unit(*pending.pop(0))
                    for j in (ni - 2, ni - 1):
                        ptj = pts.pop(j)
                        for hh in range(2):
                            nc.tensor.matmul(out=pv[hh][0:65, :],
                                             lhsT=vnat[:, b * 16 + j, :],
                                             rhs=ptj[:, hh, :],
                                             start=(j == 0), stop=(j == ni - 1))
                    # normalize: row 64 of pv[hh] is the softmax denominator
                    sums = norm.tile([1, 1024], F32, name="sums")
                    for hh in range(2):
                        nc.vector.tensor_copy(out=sums[0:1, 512 * hh:512 * hh + 512],
                                              in_=pv[hh][64:65, :])
                    rec = norm.tile([1, 1024], F32, name="rec")
                    nc.vector.reciprocal(out=rec, in_=sums)
                    rbc = norm.tile([64, 2, 512], F32, name="rbc")
                    for hh in range(2):
                        nc.gpsimd.partition_broadcast(
                            rbc[:, hh, :], rec[0:1, 512 * hh:512 * hh + 512])
                    dst = a0 if p == 0 else a1
                    for hh in range(2):
                        nc.vector.tensor_tensor(
                            out=dst[64 * hh:64 * hh + 64, tq:tq + 512],
                            in0=pv[hh][0:64, :], in1=rbc[:, hh, :], op=TT.mult)
                while pending:
                    emit_outproj_unit(*pending.pop(0))
                pending = [(tq + 128 * ci, n, (4 * ci + n) % 2)
                           for ci in range(4) for n in range(4)]
        while pending:
            emit_outproj_unit(*pending.pop(0))


_CACHE = {}


def _get_program():
    if "nc" not in _CACHE:
        _CACHE["nc"] = _build_program()
    return _CACHE["nc"]


def _get_runner():
    """Cached jitted shard_map executable over 8 cores (avoids per-call
    retrace that run_bass_kernel_spmd pays)."""
    if "runner" in _CACHE:
        return _CACHE["runner"]
    import jax
    from jax.sharding import Mesh, PartitionSpec
    from jax.experimental.shard_map import shard_map
    from concourse import bass2jax
    from concourse.bass2jax import _bass_exec_p

    bass2jax.install_neuronx_cc_hook()
    nc = _get_program()
    partition_name = nc.partition_id_tensor.name if nc.partition_id_tensor else None
    in_names, out_names, out_avals = [], [], []
    for alloc in nc.m.functions[0].allocations:
        if not isinstance(alloc, mybir.MemoryLocationSet):
            continue
        name = alloc.memorylocations[0].name
        if alloc.kind == "ExternalInput":
            if name != partition_name:
                in_names.append(name)
        elif alloc.kind == "ExternalOutput":
            out_names.append(name)
            out_avals.append(jax.core.ShapedArray(
                tuple(alloc.tensor_shape), mybir.dt.np(alloc.dtype)))
    n_params = len(in_names)
    n_outs = len(out_avals)
    all_in = list(in_names) + list(out_names)
    if partition_name is not None:
        all_in.append(partition_name)

    def _body(*args):
        operands = list(args)
        if partition_name is not None:
            operands.append(bass2jax.partition_id_tensor())
        return tuple(_bass_exec_p.bind(
            *operands,
            out_avals=tuple(out_avals),
            in_names=tuple(all_in),
            out_names=tuple(out_names),
            lowering_input_output_aliases=(),
            sim_require_finite=True,
            sim_require_nnan=True,
            nc=nc,
        ))

    devices = jax.devices()[:NCORES]
    mesh = Mesh(np.asarray(devices), ("core",))
    # xT / rope tables / mask are identical on every core: feed them
    # replicated (P()) so the host uploads one copy + on-device all-gather,
    # instead of 8 copies through the tunnel.
    in_specs = tuple(
        PartitionSpec() if n in REPLICATED else PartitionSpec("core")
        for n in in_names) + (PartitionSpec("core"),) * n_outs
    sharded = jax.jit(
        shard_map(_body, mesh=mesh,
                  in_specs=in_specs,
                  out_specs=(PartitionSpec("core"),) * n_outs,
                  check_rep=False),
        donate_argnums=tuple(range(n_params, n_params + n_outs)),
        keep_unused=True)

    from jax.sharding import NamedSharding
    rep = NamedSharding(mesh, PartitionSpec())
    shd = NamedSharding(mesh, PartitionSpec("core"))
    gather = jax.jit(lambda a: a, out_shardings=rep)   # upload-shard -> all-gather
    zeros = jax.jit(lambda: jnp.zeros((NCORES * T, D), jnp.bfloat16),
                    out_shardings=shd)
    reduce_y = jax.jit(lambda yc: yc.astype(jnp.float32).reshape(NCORES, T, D)
                       .sum(0), out_shardings=rep)
    _CACHE["runner"] = (sharded, in_names, out_names, out_avals,
                       mesh, rep, shd, gather, zeros, reduce_y)
    return _CACHE["runner"]


def _host_inputs(x, wq, wk, wv, wo):
    x = np.asarray(x, np.float32)
    wq = np.asarray(wq, np.float32)
    wk = np.asarray(wk, np.float32)
    wv = np.asarray(wv, np.float32)
    wo = np.asarray(wo, np.float32)

    xT = np.ascontiguousarray(x.reshape(T, D).T).astype(NPBF16)

    inv = 1.0 / (THETA ** (np.arange(0, HD, 2, dtype=np.float64) / HD))
    fr = np.outer(inv, np.arange(S, dtype=np.float64))   # [32, S]
    cosT = np.cos(fr).astype(NPBF16)
    sinT = np.sin(fr).astype(NPBF16)
    cos4 = np.ascontiguousarray(np.tile(cosT, (4, 1)))
    sin4 = np.ascontiguousarray(np.tile(sinT, (4, 1)))

    u = np.arange(896)[None, :]
    p = np.arange(128)[:, None]
    mask = (u >= p + 384).astype(NPBF16)

    in_maps = []
    for c in range(NCORES):
        cols_a, cols_b = [], []
        for h in range(HPC):
            base = (HPC * c + h) * HD
            cols_a.append(wq[:, base:base + 32])
            cols_b.append(wq[:, base + 32:base + 64])
        wq_c = np.concatenate(cols_a + cols_b, axis=1).astype(NPBF16)
        kb = c * HD
        wkv_c = np.concatenate(
            [wk[:, kb:kb + 32], wk[:, kb + 32:kb + 64], wv[:, kb:kb + HD]],
            axis=1).astype(NPBF16)
        wo_c = wo[c * HPC * HD:(c + 1) * HPC * HD, :].astype(NPBF16)
        in_maps.append({"xT": xT, "wq": wq_c, "wkv": wkv_c, "wo": wo_c,
                        "cos4": cos4, "sin4": sin4, "mask": mask})
    return in_maps


def _stage_inputs(in_maps):
    """Upload inputs: replicated tensors go up as 1/8 shards and are
    all-gathered on device; per-core tensors upload as the usual concat."""
    import jax
    (sharded, in_names, out_names, out_avals,
     mesh, rep, shd, gather, zeros, reduce_y) = _get_runner()
    staged = []
    for n in in_names:
        if n in REPLICATED:
            a = in_maps[0][n]
            if a.shape[0] % NCORES == 0:
                staged.append(gather(jax.device_put(a, shd)))
            else:
                staged.append(jax.device_put(a, rep))
        else:
            cat = np.concatenate([m[n] for m in in_maps], axis=0)
            staged.append(jax.device_put(cat, shd))
    return staged


def kernel(x, wq, wk, wv, wo):
    import jax
    (sharded, in_names, out_names, out_avals,
     mesh, rep, shd, gather, zeros, reduce_y) = _get_runner()
    in_maps = _host_inputs(x, wq, wk, wv, wo)
    staged = _stage_inputs(in_maps)
    out_arrs = sharded(*staged, zeros())
    ysum = reduce_y(out_arrs[out_names.index("y")])
    return np.asarray(ysum).reshape(B, S, D)


# revision 37
# speedup vs baseline: 118.7838x; 1.0546x over previous
"""Grouped-Query Attention (B=2, S=2048, D=2048, H=32, KV=8, HD=64) on 8 TRN2
NeuronCores, tensor-parallel over KV-head groups (1 KV head + 4 Q heads per
core), with host-side shard/gather.

v2: bf16 compute (halved HBM traffic, 2x DVE on 16-bit), pair-packed score
tiles with one exp per head-pair (Activation engine is the phase-2
bottleneck), PSUM retagged so scores double-buffer and the output projection
of tile jq overlaps attention of jq+1, engine rebalance (rope-K + copies on
Pool/Act, masks + normalize on DVE).

Per-core dataflow (activations feature-on-partitions; every matmul contracts
over the partition dim, no transposition of x):

  phase 1  QKV projection + RoPE, per 512-token tile
    psum: qa/qb (q ra/rb halves of 4 heads), kv = [ka|kb|v]; RoPE combines
    write q directly into qrot[128, hh, T] (head pairs in partition halves),
    k into krot rows 0:64 (replicated to 64:128 by per-tile DMA); V back to
    natural [tok, hd] via PE transpose.
  phase 2  attention per (batch, q-tile of 512), causal-block-skipped,
    head pairs processed in sequential i-loops:
      scoresT[2 x 128 kpos, 512 q] -> one exp per pair -> diag mask ->
      PV accumulate [65, 512] per head (row 64 = softmax denominator via
      ones column of V);  normalize -> a0/a1 (bf16);
    then the 512-token output projection y = [a0;a1].T @ wo on its own psum
    tags, overlapping the next q-tile's attention; y written bf16, host
    reduces the 8 row-sharded partials in f32.
"""

import contextlib
import numpy as np
import jax.numpy as jnp

import concourse.bass as bass
import concourse.tile as tile
from concourse import bacc, mybir
from concourse.masks import make_identity

B, S, D = 2, 2048, 2048
H, KV, HD = 32, 8, 64
T = B * S
NCORES = 8
HPC = H // NCORES          # 4 query heads per core
SCALE = 1.0 / np.sqrt(HD)
THETA = 10000.0
NQT = T // 512             # 8 token tiles of 512
REPLICATED = {"xT", "cos4", "sin4", "mask"}  # same bytes on every core
NDT = D // 128             # 16 contraction tiles
F32 = mybir.dt.float32
BF16 = mybir.dt.bfloat16
NPBF16 = mybir.dt.np(BF16)


def _build_program():
    nc = bacc.Bacc("TRN2", target_bir_lowering=False, debug=False)

    xT = nc.dram_tensor("xT", [D, T], BF16, kind="ExternalInput")
    wq = nc.dram_tensor("wq", [D, 2 * HPC * 32], BF16, kind="ExternalInput")
    wkv = nc.dram_tensor("wkv", [D, 128], BF16, kind="ExternalInput")
    wo = nc.dram_tensor("wo", [HPC * HD, D], BF16, kind="ExternalInput")
    cos4 = nc.dram_tensor("cos4", [128, S], BF16, kind="ExternalInput")
    sin4 = nc.dram_tensor("sin4", [128, S], BF16, kind="ExternalInput")
    maskd = nc.dram_tensor("mask", [128, 896], BF16, kind="ExternalInput")
    y = nc.dram_tensor("y", [T, D], BF16, kind="ExternalOutput")

    with tile.TileContext(nc) as tc:
        _body(tc, nc, xT, wq, wkv, wo, cos4, sin4, maskd, y)
    nc.compile()
    return nc


def _body(tc, nc, xT, wq, wkv, wo, cos4, sin4, maskd, y):
    TT = mybir.AluOpType
    EXP = mybir.ActivationFunctionType.Exp
    ctx = contextlib.ExitStack()
    with ctx:
        const = ctx.enter_context(tc.tile_pool(name="const", bufs=1))
        persist = ctx.enter_context(tc.tile_pool(name="persist", bufs=1))
        xs = ctx.enter_context(tc.tile_pool(name="xs", bufs=18))
        rtmp = ctx.enter_context(tc.tile_pool(name="rtmp", bufs=1))
        probs = ctx.enter_context(tc.tile_pool(name="probs", bufs=4))
        yout = ctx.enter_context(tc.tile_pool(name="yout", bufs=3))
        norm = ctx.enter_context(tc.tile_pool(name="norm", bufs=2))

        # PSUM: 16KB/partition, allocated exactly:
        #   T0..T3: [128,512] f32 (2KB = 1 bank each)
        #   S0,S1 : [128,2,512] f32 (4KB = 2 banks each)
        # phase 1: qa/qb even qt -> T0/T1, odd qt -> S0 halves; kv=T2, tp=T3
        # phase 2: scores alternate S0/S1; pv pair -> T2/T3; out-proj T0/T1
        psum = ctx.enter_context(tc.tile_pool(name="psum", bufs=1, space="PSUM"))

        def pT(i):
            return psum.tile([128, 512], F32, name=f"psT{i}")

        def pS(i):
            return psum.tile([128, 2, 512], F32, name=f"psS{i}")

        # ---- constants ----
        wq_sb = const.tile([128, NDT, 256], BF16, name="wq_sb")
        nc.sync.dma_start(out=wq_sb, in_=wq[:, :].rearrange("(t p) c -> p t c", p=128))
        wkv_sb = const.tile([128, NDT, 128], BF16, name="wkv_sb")
        nc.sync.dma_start(out=wkv_sb, in_=wkv[:, :].rearrange("(t p) c -> p t c", p=128))
        wo_sb = const.tile([128, 2, D], BF16, name="wo_sb")
        nc.sync.dma_start(out=wo_sb, in_=wo[:, :].rearrange("(t p) c -> p t c", p=128))
        cos_sb = const.tile([128, S], BF16, name="cos_sb")
        nc.sync.dma_start(out=cos_sb, in_=cos4[:, :])
        sin_sb = const.tile([128, S], BF16, name="sin_sb")
        nc.sync.dma_start(out=sin_sb, in_=sin4[:, :])
        mask_sb = const.tile([128, 896], BF16, name="mask_sb")
        nc.sync.dma_start(out=mask_sb, in_=maskd[:, :])
        ident = const.tile([64, 64], F32, name="ident")
        make_identity(nc, ident)

        # ---- persistent activations ----
        # qrot: [64*pair + (0:32 ra | 32:64 rb), head-in-pair, tok]
        qrot = persist.tile([128, 2, T], BF16, name="qrot")
        krot = persist.tile([128, T], BF16, name="krot")   # 64:128 = replica
        vnat = persist.tile([128, T // 128, 65], BF16, name="vnat")
        a0 = persist.tile([128, T], BF16, name="a0")       # heads 0,1
        a1 = persist.tile([128, T], BF16, name="a1")       # heads 2,3
        ones_c = const.tile([128, T // 128, 1], F32, name="ones_c")
        nc.vector.memset(ones_c, 1.0)
        nc.vector.tensor_copy(out=vnat[:, :, 64:65], in_=ones_c)

        # ================= phase 1: projections + rope =================
        for qt in range(NQT):
            pos0 = (qt % 4) * 512
            tok0 = qt * 512
            if qt % 2 == 0:
                qa_ps, qb_ps = pT(0), pT(1)
            else:
                s_ps = pS(0)
                qa_ps, qb_ps = s_ps[:, 0, :], s_ps[:, 1, :]
            kv_ps = pT(2)
            # kv projection first: the single-buffered kv bank is consumed by
            # rope-K (DVE) while PE continues with the qa/qb matmuls below.
            xts = []
            for d in range(NDT):
                xt = xs.tile([128, 512], BF16, name="xt")
                nc.sync.dma_start(out=xt, in_=xT[d * 128:(d + 1) * 128, tok0:tok0 + 512])
                xts.append(xt)
                nc.tensor.matmul(out=kv_ps, lhsT=wkv_sb[:, d, :], rhs=xt,
                                 start=d == 0, stop=d == NDT - 1)
            cs = cos_sb[:, pos0:pos0 + 512]
            sn = sin_sb[:, pos0:pos0 + 512]
            # stage k through SBUF bf16 (Act copy, idle engine) so every rope
            # TT runs in the DVE 16-bit fast mode
            kk = rtmp.tile([64, 512], BF16, name="kk")
            nc.scalar.copy(out=kk, in_=kv_ps[0:64])
            k_x = rtmp.tile([32, 512], BF16, name="k_x")
            k_x2 = rtmp.tile([32, 512], BF16, name="k_x2")
            k_y = rtmp.tile([32, 512], BF16, name="k_y")
            k_y2 = rtmp.tile([32, 512], BF16, name="k_y2")
            nc.vector.tensor_tensor(out=k_x, in0=kk[0:32], in1=cs[0:32], op=TT.mult)
            nc.vector.tensor_tensor(out=k_x2, in0=kk[0:32], in1=sn[0:32], op=TT.mult)
            # cos/sin tables are 4x-tiled over partitions, so rows 32:64
            # equal rows 0:32; index them to satisfy the equal-base-partition
            # rule for SBUF-SBUF TensorTensor.
            nc.vector.tensor_tensor(out=k_y, in0=kk[32:64], in1=sn[32:64], op=TT.mult)
            nc.vector.tensor_tensor(out=k_y2, in0=kk[32:64], in1=cs[32:64], op=TT.mult)
            vt = rtmp.tile([64, 512], F32, name="vt")
            nc.scalar.copy(out=vt, in_=kv_ps[64:128])
            nc.vector.tensor_tensor(out=krot[0:32, tok0:tok0 + 512], in0=k_x,
                                    in1=k_y, op=TT.subtract)
            nc.vector.tensor_tensor(out=krot[32:64, tok0:tok0 + 512], in0=k_x2,
                                    in1=k_y2, op=TT.add)
            nc.sync.dma_start(out=krot[64:128, tok0:tok0 + 512],
                              in_=krot[0:64, tok0:tok0 + 512])
            for d in range(NDT):
                st, sp = d == 0, d == NDT - 1
                nc.tensor.matmul(out=qa_ps, lhsT=wq_sb[:, d, 0:128], rhs=xts[d],
                                 start=st, stop=sp)
                nc.tensor.matmul(out=qb_ps, lhsT=wq_sb[:, d, 128:256], rhs=xts[d],
                                 start=st, stop=sp)
            # V -> natural [tok, hd] via PE transpose into T3, Pool copy out
            tp = pT(3)
            for k4 in range(4):
                nc.tensor.transpose(tp[:, 64 * k4:64 * k4 + 64],
                                    vt[:, 128 * k4:128 * k4 + 128], ident)
            for k4 in range(4):
                nc.scalar.copy(out=vnat[:, qt * 4 + k4, 0:64],
                               in_=tp[:, 64 * k4:64 * k4 + 64])
            # Q rope on DVE: [128, 512] (row 32h+r = head h, ra/rb dim r);
            # staged to SBUF bf16 by Act copies for the DVE fast mode
            qa_sb = rtmp.tile([128, 512], BF16, name="qa_sb")
            qb_sb = rtmp.tile([128, 512], BF16, name="qb_sb")
            nc.scalar.copy(out=qa_sb, in_=qa_ps)
            nc.scalar.copy(out=qb_sb, in_=qb_ps)
            t_x = rtmp.tile([128, 512], BF16, name="t_x")
            t_x2 = rtmp.tile([128, 512], BF16, name="t_x2")
            t_y = rtmp.tile([128, 512], BF16, name="t_y")
            t_y2 = rtmp.tile([128, 512], BF16, name="t_y2")
            nc.vector.tensor_tensor(out=t_x, in0=qa_sb, in1=cs, op=TT.mult)
            nc.vector.tensor_tensor(out=t_x2, in0=qa_sb, in1=sn, op=TT.mult)
            nc.vector.tensor_tensor(out=t_y, in0=qb_sb, in1=sn, op=TT.mult)
            nc.vector.tensor_tensor(out=t_y2, in0=qb_sb, in1=cs, op=TT.mult)
            for h in range(HPC):
                p, hh = h // 2, h % 2
                r0 = 32 * h
                nc.vector.tensor_tensor(
                    out=qrot[64 * p:64 * p + 32, hh, tok0:tok0 + 512],
                    in0=t_x[r0:r0 + 32], in1=t_y[r0:r0 + 32], op=TT.subtract)
                nc.vector.tensor_tensor(
                    out=qrot[64 * p + 32:64 * p + 64, hh, tok0:tok0 + 512],
                    in0=t_x2[r0:r0 + 32], in1=t_y2[r0:r0 + 32], op=TT.add)

        # ================= phase 2: attention + out-proj =================
        # Out-projection of q-tile jq runs on its own psum tags (T0/T1),
        # drained one 2-matmul unit per attention iteration of the NEXT
        # q-tile so the in-order PE queue never stalls the exp stream.
        def emit_outproj_unit(ts, n, par):
            yo = pT(par)
            nc.tensor.matmul(out=yo, lhsT=a0[:, ts:ts + 128],
                             rhs=wo_sb[:, 0, 512 * n:512 * n + 512],
                             start=True, stop=False)
            nc.tensor.matmul(out=yo, lhsT=a1[:, ts:ts + 128],
                             rhs=wo_sb[:, 1, 512 * n:512 * n + 512],
                             start=False, stop=True)
            ys = yout.tile([128, 512], BF16, name="ys")
            nc.vector.tensor_copy(out=ys, in_=yo)
            nc.sync.dma_start(out=y[ts:ts + 128, 512 * n:512 * n + 512], in_=ys)

        pending = []
        for b in range(B):
            for jq in range(4):
                tq = b * S + jq * 512
                ni = 4 * jq + 4
                for p in range(2):  # head pair: heads 2p, 2p+1
                    pv = [pT(2), pT(3)]
                    # 2-ahead software pipeline: PE queue per i is
                    # [scores(i), pv(i-2), outproj-unit] so exp(i-1) is never
                    # behind a matmul that waits on it.
                    pts = {}
                    for i in range(ni):
                        tk = b * S + i * 128
                        sc = pS(i % 2)
                        for hh in range(2):
                            nc.tensor.matmul(
                                out=sc[:, hh, :],
                                lhsT=krot[64 * p:64 * p + 64, tk:tk + 128],
                                rhs=qrot[64 * p:64 * p + 64, hh, tq:tq + 512],
                                start=True, stop=True)
                        pt = probs.tile([128, 2, 512], BF16, name="pt")
                        nc.scalar.activation(out=pt, in_=sc, func=EXP,
                                             scale=float(SCALE))
                        if i >= 4 * jq:  # diagonal block: causal mask
                            roff = 128 * i - 512 * jq
                            for hh in range(2):
                                nc.vector.tensor_tensor(
                                    out=pt[:, hh, :], in0=pt[:, hh, :],
                                    in1=mask_sb[:, 384 - roff:896 - roff],
                                    op=TT.mult)
                        pts[i] = pt
                        if i >= 2:
                            j = i - 2
                            ptj = pts.pop(j)
                            for hh in range(2):
                                nc.tensor.matmul(out=pv[hh][0:65, :],
                                                 lhsT=vnat[:, b * 16 + j, :],
                                                 rhs=ptj[:, hh, :],
                                                 start=(j == 0), stop=False)
                        if pending:
                            emit_outproj_unit(*pending.pop(0))
                    for j in (ni - 2, ni - 1):
                        ptj = pts.pop(j)
                        for hh in range(2):
                            nc.tensor.matmul(out=pv[hh][0:65, :],
                                             lhsT=vnat[:, b * 16 + j, :],
                                             rhs=ptj[:, hh, :],
                                             start=(j == 0), stop=(j == ni - 1))
                    # normalize: row 64 of pv[hh] is the softmax denominator
                    sums = norm.tile([1, 1024], F32, name="sums")
                    for hh in range(2):
                        nc.scalar.copy(out=sums[0:1, 512 * hh:512 * hh + 512],
                                       in_=pv[hh][64:65, :])
                    rec = norm.tile([1, 1024], F32, name="rec")
                    nc.vector.reciprocal(out=rec, in_=sums)
                    rbc = norm.tile([64, 2, 512], F32, name="rbc")
                    for hh in range(2):
                        nc.gpsimd.partition_broadcast(
                            rbc[:, hh, :], rec[0:1, 512 * hh:512 * hh + 512])
                    dst = a0 if p == 0 else a1
                    for hh in range(2):
                        nc.vector.tensor_tensor(
                            out=dst[64 * hh:64 * hh + 64, tq:tq + 512],
                            in0=pv[hh][0:64, :], in1=rbc[:, hh, :], op=TT.mult)
                while pending:
                    emit_outproj_unit(*pending.pop(0))
                pending = [(tq + 128 * ci, n, (4 * ci + n) % 2)
                           for ci in range(4) for n in range(4)]
        while pending:
            emit_outproj_unit(*pending.pop(0))


_CACHE = {}


def _get_program():
    if "nc" not in _CACHE:
        _CACHE["nc"] = _build_program()
    return _CACHE["nc"]


def _get_runner():
    """Cached jitted shard_map executable over 8 cores (avoids per-call
    retrace that run_bass_kernel_spmd pays)."""
    if "runner" in _CACHE:
        return _CACHE["runner"]
    import jax
    from jax.sharding import Mesh, PartitionSpec
    from jax.experimental.shard_map import shard_map
    from concourse import bass2jax
    from concourse.bass2jax import _bass_exec_p

    bass2jax.install_neuronx_cc_hook()
    nc = _get_program()
    partition_name = nc.partition_id_tensor.name if nc.partition_id_tensor else None
    in_names, out_names, out_avals = [], [], []
    for alloc in nc.m.functions[0].allocations:
        if not isinstance(alloc, mybir.MemoryLocationSet):
            continue
        name = alloc.memorylocations[0].name
        if alloc.kind == "ExternalInput":
            if name != partition_name:
                in_names.append(name)
        elif alloc.kind == "ExternalOutput":
            out_names.append(name)
            out_avals.append(jax.core.ShapedArray(
                tuple(alloc.tensor_shape), mybir.dt.np(alloc.dtype)))
    n_params = len(in_names)
    n_outs = len(out_avals)
    all_in = list(in_names) + list(out_names)
    if partition_name is not None:
        all_in.append(partition_name)

    def _body(*args):
        operands = list(args)
        if partition_name is not None:
            operands.append(bass2jax.partition_id_tensor())
        return tuple(_bass_exec_p.bind(
            *operands,
            out_avals=tuple(out_avals),
            in_names=tuple(all_in),
            out_names=tuple(out_names),
            lowering_input_output_aliases=(),
            sim_require_finite=True,
            sim_require_nnan=True,
            nc=nc,
        ))

    devices = jax.devices()[:NCORES]
    mesh = Mesh(np.asarray(devices), ("core",))
    # xT / rope tables / mask are identical on every core: feed them
    # replicated (P()) so the host uploads one copy + on-device all-gather,
    # instead of 8 copies through the tunnel.
    in_specs = tuple(
        PartitionSpec() if n in REPLICATED else PartitionSpec("core")
        for n in in_names) + (PartitionSpec("core"),) * n_outs
    sharded = jax.jit(
        shard_map(_body, mesh=mesh,
                  in_specs=in_specs,
                  out_specs=(PartitionSpec("core"),) * n_outs,
                  check_rep=False),
        donate_argnums=tuple(range(n_params, n_params + n_outs)),
        keep_unused=True)

    from jax.sharding import NamedSharding
    rep = NamedSharding(mesh, PartitionSpec())
    shd = NamedSharding(mesh, PartitionSpec("core"))
    gather = jax.jit(lambda a: a, out_shardings=rep)   # upload-shard -> all-gather
    zeros = jax.jit(lambda: jnp.zeros((NCORES * T, D), jnp.bfloat16),
                    out_shardings=shd)
    reduce_y = jax.jit(lambda yc: yc.astype(jnp.float32).reshape(NCORES, T, D)
                       .sum(0), out_shardings=rep)
    _CACHE["runner"] = (sharded, in_names, out_names, out_avals,
                       mesh, rep, shd, gather, zeros, reduce_y)
    return _CACHE["runner"]


def _host_inputs(x, wq, wk, wv, wo):
    x = np.asarray(x, np.float32)
    wq = np.asarray(wq, np.float32)
    wk = np.asarray(wk, np.float32)
    wv = np.asarray(wv, np.float32)
    wo = np.asarray(wo, np.float32)

    xT = np.ascontiguousarray(x.reshape(T, D).T).astype(NPBF16)

    inv = 1.0 / (THETA ** (np.arange(0, HD, 2, dtype=np.float64) / HD))
    fr = np.outer(inv, np.arange(S, dtype=np.float64))   # [32, S]
    cosT = np.cos(fr).astype(NPBF16)
    sinT = np.sin(fr).astype(NPBF16)
    cos4 = np.ascontiguousarray(np.tile(cosT, (4, 1)))
    sin4 = np.ascontiguousarray(np.tile(sinT, (4, 1)))

    u = np.arange(896)[None, :]
    p = np.arange(128)[:, None]
    mask = (u >= p + 384).astype(NPBF16)

    in_maps = []
    for c in range(NCORES):
        cols_a, cols_b = [], []
        for h in range(HPC):
            base = (HPC * c + h) * HD
            cols_a.append(wq[:, base:base + 32])
            cols_b.append(wq[:, base + 32:base + 64])
        wq_c = np.concatenate(cols_a + cols_b, axis=1).astype(NPBF16)
        kb = c * HD
        wkv_c = np.concatenate(
            [wk[:, kb:kb + 32], wk[:, kb + 32:kb + 64], wv[:, kb:kb + HD]],
            axis=1).astype(NPBF16)
        wo_c = wo[c * HPC * HD:(c + 1) * HPC * HD, :].astype(NPBF16)
        in_maps.append({"xT": xT, "wq": wq_c, "wkv": wkv_c, "wo": wo_c,
                        "cos4": cos4, "sin4": sin4, "mask": mask})
    return in_maps


def _stage_inputs(in_maps):
    """Upload inputs: replicated tensors go up as 1/8 shards and are
    all-gathered on device; per-core tensors upload as the usual concat."""
    import jax
    (sharded, in_names, out_names, out_avals,
     mesh, rep, shd, gather, zeros, reduce_y) = _get_runner()
    staged = []
    for n in in_names:
        if n in REPLICATED:
            a = in_maps[0][n]
            if a.shape[0] % NCORES == 0:
                staged.append(gather(jax.device_put(a, shd)))
            else:
                staged.append(jax.device_put(a, rep))
        else:
            cat = np.concatenate([m[n] for m in in_maps], axis=0)
            staged.append(jax.device_put(cat, shd))
    return staged


def kernel(x, wq, wk, wv, wo):
    import jax
    (sharded, in_names, out_names, out_avals,
     mesh, rep, shd, gather, zeros, reduce_y) = _get_runner()
    in_maps = _host_inputs(x, wq, wk, wv, wo)
    staged = _stage_inputs(in_maps)
    out_arrs = sharded(*staged, zeros())
    ysum = reduce_y(out_arrs[out_names.index("y")])
    return np.asarray(ysum).reshape(B, S, D)


# revision 46
# speedup vs baseline: 132.0388x; 1.1116x over previous
"""Grouped-Query Attention (B=2, S=2048, D=2048, H=32, KV=8, HD=64) on 8 TRN2
NeuronCores, tensor-parallel over KV-head groups (1 KV head + 4 Q heads per
core), with host-side shard/gather.

v2: bf16 compute (halved HBM traffic, 2x DVE on 16-bit), pair-packed score
tiles with one exp per head-pair (Activation engine is the phase-2
bottleneck), PSUM retagged so scores double-buffer and the output projection
of tile jq overlaps attention of jq+1, engine rebalance (rope-K + copies on
Pool/Act, masks + normalize on DVE).

Per-core dataflow (activations feature-on-partitions; every matmul contracts
over the partition dim, no transposition of x):

  phase 1  QKV projection + RoPE, per 512-token tile
    psum: qa/qb (q ra/rb halves of 4 heads), kv = [ka|kb|v]; RoPE combines
    write q directly into qrot[128, hh, T] (head pairs in partition halves),
    k into krot rows 0:64 (replicated to 64:128 by per-tile DMA); V back to
    natural [tok, hd] via PE transpose.
  phase 2  attention per (batch, q-tile of 512), causal-block-skipped,
    head pairs processed in sequential i-loops:
      scoresT[2 x 128 kpos, 512 q] -> one exp per pair -> diag mask ->
      PV accumulate [65, 512] per head (row 64 = softmax denominator via
      ones column of V);  normalize -> a0/a1 (bf16);
    then the 512-token output projection y = [a0;a1].T @ wo on its own psum
    tags, overlapping the next q-tile's attention; y written bf16, host
    reduces the 8 row-sharded partials in f32.
"""

import contextlib
import numpy as np
import jax.numpy as jnp

import concourse.bass as bass
import concourse.tile as tile
from concourse import bacc, mybir
from concourse.masks import make_identity

B, S, D = 2, 2048, 2048
H, KV, HD = 32, 8, 64
T = B * S
NCORES = 8
HPC = H // NCORES          # 4 query heads per core
SCALE = 1.0 / np.sqrt(HD)
THETA = 10000.0
NQT = T // 512             # 8 token tiles of 512
REPLICATED = {"xT", "cos4", "sin4", "mask"}  # same bytes on every core
NDT = D // 128             # 16 contraction tiles
F32 = mybir.dt.float32
BF16 = mybir.dt.bfloat16
NPBF16 = mybir.dt.np(BF16)


def _build_program():
    nc = bacc.Bacc("TRN2", target_bir_lowering=False, debug=False)

    xT = nc.dram_tensor("xT", [D, T], BF16, kind="ExternalInput")
    wq = nc.dram_tensor("wq", [D, 2 * HPC * 32], BF16, kind="ExternalInput")
    wkv = nc.dram_tensor("wkv", [D, 128], BF16, kind="ExternalInput")
    wo = nc.dram_tensor("wo", [HPC * HD, D], BF16, kind="ExternalInput")
    cos4 = nc.dram_tensor("cos4", [128, S], BF16, kind="ExternalInput")
    sin4 = nc.dram_tensor("sin4", [128, S], BF16, kind="ExternalInput")
    maskd = nc.dram_tensor("mask", [128, 896], BF16, kind="ExternalInput")
    y = nc.dram_tensor("y", [T, D], BF16, kind="ExternalOutput")

    with tile.TileContext(nc) as tc:
        _body(tc, nc, xT, wq, wkv, wo, cos4, sin4, maskd, y)
    nc.compile()
    return nc


def _body(tc, nc, xT, wq, wkv, wo, cos4, sin4, maskd, y):
    TT = mybir.AluOpType
    EXP = mybir.ActivationFunctionType.Exp
    ctx = contextlib.ExitStack()
    with ctx:
        const = ctx.enter_context(tc.tile_pool(name="const", bufs=1))
        persist = ctx.enter_context(tc.tile_pool(name="persist", bufs=1))
        xs = ctx.enter_context(tc.tile_pool(name="xs", bufs=9))
        rtmp = ctx.enter_context(tc.tile_pool(name="rtmp", bufs=1))
        probs = ctx.enter_context(tc.tile_pool(name="probs", bufs=6))
        yout = ctx.enter_context(tc.tile_pool(name="yout", bufs=3))
        norm = ctx.enter_context(tc.tile_pool(name="norm", bufs=2))

        # PSUM: 16KB/partition, allocated exactly:
        #   T0..T3: [128,512] f32 (2KB = 1 bank each)
        #   S0,S1 : [128,2,512] f32 (4KB = 2 banks each)
        # phase 1: qa/qb even qt -> T0/T1, odd qt -> S0 halves; kv=T2, tp=T3
        # phase 2: scores alternate S0/S1; pv pair -> T2/T3; out-proj T0/T1
        psum = ctx.enter_context(tc.tile_pool(name="psum", bufs=1, space="PSUM"))

        def pT(i):
            return psum.tile([128, 512], F32, name=f"psT{i}")

        def pS(i):
            return psum.tile([128, 2, 512], F32, name=f"psS{i}")

        # ---- constants ----
        # load order = first-use order: wkv gates the very first matmul,
        # wq the first q matmuls, wo is not needed until phase 2.
        wkv_sb = const.tile([128, NDT, 128], BF16, name="wkv_sb")
        wq_sb = const.tile([128, NDT, 256], BF16, name="wq_sb")
        # quarter-wise loads: the first kv/q matmuls only need the first
        # d-tiles, so they start before the full weight DMA lands
        for q4 in range(4):
            dl = slice(q4 * NDT // 4, (q4 + 1) * NDT // 4)
            dr = slice(q4 * (D // 4), (q4 + 1) * (D // 4))
            nc.sync.dma_start(out=wkv_sb[:, dl, :],
                              in_=wkv[dr, :].rearrange("(t p) c -> p t c", p=128))
            nc.sync.dma_start(out=wq_sb[:, dl, :],
                              in_=wq[dr, :].rearrange("(t p) c -> p t c", p=128))
        cos_sb = const.tile([128, S], BF16, name="cos_sb")
        nc.sync.dma_start(out=cos_sb, in_=cos4[:, :])
        sin_sb = const.tile([128, S], BF16, name="sin_sb")
        nc.sync.dma_start(out=sin_sb, in_=sin4[:, :])
        mask_sb = const.tile([128, 896], BF16, name="mask_sb")
        nc.sync.dma_start(out=mask_sb, in_=maskd[:, :])
        wo_sb = const.tile([128, 2, D], BF16, name="wo_sb")
        nc.sync.dma_start(out=wo_sb, in_=wo[:, :].rearrange("(t p) c -> p t c", p=128))
        ident = const.tile([64, 64], F32, name="ident")
        make_identity(nc, ident)

        # ---- persistent activations ----
        # qrot: [64*pair + (0:32 ra | 32:64 rb), head-in-pair, tok]
        qrot = persist.tile([128, 2, T], BF16, name="qrot")
        krot = persist.tile([128, T], BF16, name="krot")   # 64:128 = replica
        vnat = persist.tile([128, T // 128, 65], BF16, name="vnat")
        a0 = persist.tile([128, T], BF16, name="a0")       # heads 0,1
        a1 = persist.tile([128, T], BF16, name="a1")       # heads 2,3
        ones_c = const.tile([128, T // 128, 1], F32, name="ones_c")
        nc.vector.memset(ones_c, 1.0)
        nc.vector.tensor_copy(out=vnat[:, :, 64:65], in_=ones_c)

        # ================= phase 1: projections + rope =================
        for qt in range(NQT):
            pos0 = (qt % 4) * 512
            tok0 = qt * 512
            if qt % 2 == 0:
                qa_ps, qb_ps = pT(0), pT(1)
            else:
                s_ps = pS(0)
                qa_ps, qb_ps = s_ps[:, 0, :], s_ps[:, 1, :]
            kv_ps = pT(2)
            # kv projection first: the single-buffered kv bank is consumed by
            # rope-K (DVE) while PE continues with the qa/qb matmuls below.
            # x tiles arrive 2 contraction blocks per DMA (fewer, larger
            # transfers -> half the HWDGE queue work).
            xts = []
            for d2 in range(NDT // 2):
                xt = xs.tile([128, 2, 512], BF16, name="xt")
                nc.sync.dma_start(
                    out=xt,
                    in_=xT[d2 * 256:(d2 + 1) * 256, tok0:tok0 + 512]
                    .rearrange("(g p) t -> p g t", p=128))
                xts.append(xt)
                for g in range(2):
                    d = 2 * d2 + g
                    nc.tensor.matmul(out=kv_ps, lhsT=wkv_sb[:, d, :],
                                     rhs=xt[:, g, :],
                                     start=d == 0, stop=d == NDT - 1)
            cs = cos_sb[:, pos0:pos0 + 512]
            sn = sin_sb[:, pos0:pos0 + 512]
            # stage k through SBUF bf16 (Act copy, idle engine) so every rope
            # TT runs in the DVE 16-bit fast mode
            kk = rtmp.tile([64, 512], BF16, name="kk")
            nc.scalar.copy(out=kk, in_=kv_ps[0:64])
            k_x = rtmp.tile([32, 512], BF16, name="k_x")
            k_x2 = rtmp.tile([32, 512], BF16, name="k_x2")
            k_y = rtmp.tile([32, 512], BF16, name="k_y")
            k_y2 = rtmp.tile([32, 512], BF16, name="k_y2")
            nc.vector.tensor_tensor(out=k_x, in0=kk[0:32], in1=cs[0:32], op=TT.mult)
            nc.vector.tensor_tensor(out=k_x2, in0=kk[0:32], in1=sn[0:32], op=TT.mult)
            # cos/sin tables are 4x-tiled over partitions, so rows 32:64
            # equal rows 0:32; index them to satisfy the equal-base-partition
            # rule for SBUF-SBUF TensorTensor.
            nc.vector.tensor_tensor(out=k_y, in0=kk[32:64], in1=sn[32:64], op=TT.mult)
            nc.vector.tensor_tensor(out=k_y2, in0=kk[32:64], in1=cs[32:64], op=TT.mult)
            vt = rtmp.tile([64, 512], F32, name="vt")
            nc.scalar.copy(out=vt, in_=kv_ps[64:128])
            nc.vector.tensor_tensor(out=krot[0:32, tok0:tok0 + 512], in0=k_x,
                                    in1=k_y, op=TT.subtract)
            nc.vector.tensor_tensor(out=krot[32:64, tok0:tok0 + 512], in0=k_x2,
                                    in1=k_y2, op=TT.add)
            nc.sync.dma_start(out=krot[64:128, tok0:tok0 + 512],
                              in_=krot[0:64, tok0:tok0 + 512])
            for d in range(NDT):
                st, sp = d == 0, d == NDT - 1
                xtg = xts[d // 2][:, d % 2, :]
                nc.tensor.matmul(out=qa_ps, lhsT=wq_sb[:, d, 0:128], rhs=xtg,
                                 start=st, stop=sp)
                nc.tensor.matmul(out=qb_ps, lhsT=wq_sb[:, d, 128:256], rhs=xtg,
                                 start=st, stop=sp)
            # V -> natural [tok, hd] via PE transpose into T3, Pool copy out
            tp = pT(3)
            for k4 in range(4):
                nc.tensor.transpose(tp[:, 64 * k4:64 * k4 + 64],
                                    vt[:, 128 * k4:128 * k4 + 128], ident)
            for k4 in range(4):
                nc.scalar.copy(out=vnat[:, qt * 4 + k4, 0:64],
                               in_=tp[:, 64 * k4:64 * k4 + 64])
            # Q rope on DVE: [128, 512] (row 32h+r = head h, ra/rb dim r);
            # staged to SBUF bf16 by Act copies for the DVE fast mode
            qa_sb = rtmp.tile([128, 512], BF16, name="qa_sb")
            qb_sb = rtmp.tile([128, 512], BF16, name="qb_sb")
            nc.scalar.copy(out=qa_sb, in_=qa_ps)
            nc.scalar.copy(out=qb_sb, in_=qb_ps)
            t_x = rtmp.tile([128, 512], BF16, name="t_x")
            t_x2 = rtmp.tile([128, 512], BF16, name="t_x2")
            t_y = rtmp.tile([128, 512], BF16, name="t_y")
            t_y2 = rtmp.tile([128, 512], BF16, name="t_y2")
            nc.vector.tensor_tensor(out=t_x, in0=qa_sb, in1=cs, op=TT.mult)
            nc.vector.tensor_tensor(out=t_x2, in0=qa_sb, in1=sn, op=TT.mult)
            nc.vector.tensor_tensor(out=t_y, in0=qb_sb, in1=sn, op=TT.mult)
            nc.vector.tensor_tensor(out=t_y2, in0=qb_sb, in1=cs, op=TT.mult)
            for h in range(HPC):
                p, hh = h // 2, h % 2
                r0 = 32 * h
                nc.vector.tensor_tensor(
                    out=qrot[64 * p:64 * p + 32, hh, tok0:tok0 + 512],
                    in0=t_x[r0:r0 + 32], in1=t_y[r0:r0 + 32], op=TT.subtract)
                nc.vector.tensor_tensor(
                    out=qrot[64 * p + 32:64 * p + 64, hh, tok0:tok0 + 512],
                    in0=t_x2[r0:r0 + 32], in1=t_y2[r0:r0 + 32], op=TT.add)

        # ================= phase 2: attention + out-proj =================
        # Out-projection of q-tile jq runs on its own psum tags (T0/T1),
        # drained one 2-matmul unit per attention iteration of the NEXT
        # q-tile so the in-order PE queue never stalls the exp stream.
        def emit_outproj_unit(ts, n, par):
            yo = pT(par)
            nc.tensor.matmul(out=yo, lhsT=a0[:, ts:ts + 128],
                             rhs=wo_sb[:, 0, 512 * n:512 * n + 512],
                             start=True, stop=False)
            nc.tensor.matmul(out=yo, lhsT=a1[:, ts:ts + 128],
                             rhs=wo_sb[:, 1, 512 * n:512 * n + 512],
                             start=False, stop=True)
            ys = yout.tile([128, 512], BF16, name="ys")
            nc.vector.tensor_copy(out=ys, in_=yo)
            nc.sync.dma_start(out=y[ts:ts + 128, 512 * n:512 * n + 512], in_=ys)

        pending = []
        for b in range(B):
            for jq in range(4):
                tq = b * S + jq * 512
                ni = 4 * jq + 4
                for p in range(2):  # head pair: heads 2p, 2p+1
                    pv = [pT(2), pT(3)]
                    # 2-ahead software pipeline: PE queue per i is
                    # [scores(i), pv(i-2), outproj-unit] so exp(i-1) is never
                    # behind a matmul that waits on it.
                    pts = {}
                    for i in range(ni):
                        tk = b * S + i * 128
                        sc = pS(i % 2)
                        for hh in range(2):
                            nc.tensor.matmul(
                                out=sc[:, hh, :],
                                lhsT=krot[64 * p:64 * p + 64, tk:tk + 128],
                                rhs=qrot[64 * p:64 * p + 64, hh, tq:tq + 512],
                                start=True, stop=True)
                        pt = probs.tile([128, 2, 512], BF16, name="pt")
                        nc.scalar.activation(out=pt, in_=sc, func=EXP,
                                             scale=float(SCALE))
                        if i >= 4 * jq:  # diagonal block: causal mask
                            roff = 128 * i - 512 * jq
                            for hh in range(2):
                                nc.vector.tensor_tensor(
                                    out=pt[:, hh, :], in0=pt[:, hh, :],
                                    in1=mask_sb[:, 384 - roff:896 - roff],
                                    op=TT.mult)
                        pts[i] = pt
                        if i >= 3:
                            j = i - 3
                            ptj = pts.pop(j)
                            for hh in range(2):
                                nc.tensor.matmul(out=pv[hh][0:65, :],
                                                 lhsT=vnat[:, b * 16 + j, :],
                                                 rhs=ptj[:, hh, :],
                                                 start=(j == 0), stop=False)
                        if pending:
                            emit_outproj_unit(*pending.pop(0))
                    for j in (ni - 3, ni - 2, ni - 1):
                        ptj = pts.pop(j)
                        for hh in range(2):
                            nc.tensor.matmul(out=pv[hh][0:65, :],
                                             lhsT=vnat[:, b * 16 + j, :],
                                             rhs=ptj[:, hh, :],
                                             start=(j == 0), stop=(j == ni - 1))
                    # normalize: row 64 of pv[hh] is the softmax denominator
                    sums = norm.tile([1, 1024], F32, name="sums")
                    for hh in range(2):
                        nc.scalar.copy(out=sums[0:1, 512 * hh:512 * hh + 512],
                                       in_=pv[hh][64:65, :])
                    rec = norm.tile([1, 1024], F32, name="rec")
                    nc.vector.reciprocal(out=rec, in_=sums)
                    rbc = norm.tile([64, 2, 512], F32, name="rbc")
                    for hh in range(2):
                        nc.gpsimd.partition_broadcast(
                            rbc[:, hh, :], rec[0:1, 512 * hh:512 * hh + 512])
                    dst = a0 if p == 0 else a1
                    for hh in range(2):
                        nc.vector.tensor_tensor(
                            out=dst[64 * hh:64 * hh + 64, tq:tq + 512],
                            in0=pv[hh][0:64, :], in1=rbc[:, hh, :], op=TT.mult)
                while pending:
                    emit_outproj_unit(*pending.pop(0))
                pending = [(tq + 128 * ci, n, (4 * ci + n) % 2)
                           for ci in range(4) for n in range(4)]
        while pending:
            emit_outproj_unit(*pending.pop(0))


_CACHE = {}


def _get_program():
    if "nc" not in _CACHE:
        _CACHE["nc"] = _build_program()
    return _CACHE["nc"]


def _get_runner():
    """Cached jitted shard_map executable over 8 cores (avoids per-call
    retrace that run_bass_kernel_spmd pays)."""
    if "runner" in _CACHE:
        return _CACHE["runner"]
    import jax
    from jax.sharding import Mesh, PartitionSpec
    from jax.experimental.shard_map import shard_map
    from concourse import bass2jax
    from concourse.bass2jax import _bass_exec_p

    bass2jax.install_neuronx_cc_hook()
    nc = _get_program()
    partition_name = nc.partition_id_tensor.name if nc.partition_id_tensor else None
    in_names, out_names, out_avals, in_shapes = [], [], [], {}
    for alloc in nc.m.functions[0].allocations:
        if not isinstance(alloc, mybir.MemoryLocationSet):
            continue
        name = alloc.memorylocations[0].name
        if alloc.kind == "ExternalInput":
            if name != partition_name:
                in_names.append(name)
                in_shapes[name] = (tuple(alloc.tensor_shape),
                                   mybir.dt.np(alloc.dtype))
        elif alloc.kind == "ExternalOutput":
            out_names.append(name)
            out_avals.append(jax.core.ShapedArray(
                tuple(alloc.tensor_shape), mybir.dt.np(alloc.dtype)))
    n_params = len(in_names)
    n_outs = len(out_avals)
    all_in = list(in_names) + list(out_names)
    if partition_name is not None:
        all_in.append(partition_name)

    def _body(*args):
        operands = list(args)
        if partition_name is not None:
            operands.append(bass2jax.partition_id_tensor())
        return tuple(_bass_exec_p.bind(
            *operands,
            out_avals=tuple(out_avals),
            in_names=tuple(all_in),
            out_names=tuple(out_names),
            lowering_input_output_aliases=(),
            sim_require_finite=True,
            sim_require_nnan=True,
            nc=nc,
        ))

    devices = jax.devices()[:NCORES]
    mesh = Mesh(np.asarray(devices), ("core",))
    # xT / rope tables / mask are identical on every core: feed them
    # replicated (P()) so the host uploads one copy + on-device all-gather,
    # instead of 8 copies through the tunnel.
    in_specs = tuple(
        PartitionSpec() if n in REPLICATED else PartitionSpec("core")
        for n in in_names) + (PartitionSpec("core"),) * n_outs

    from jax.sharding import NamedSharding
    rep = NamedSharding(mesh, PartitionSpec())
    shd = NamedSharding(mesh, PartitionSpec("core"))

    # AOT-compile with the bass effect suppressed (C++ fast-path dispatch):
    # per-call Python jit dispatch through the tunnel costs ~0.5ms, several
    # times the kernel's own execution time.
    arg_structs = []
    for n in in_names:
        shape, dtype = in_shapes[n]
        if n in REPLICATED:
            arg_structs.append(jax.ShapeDtypeStruct(shape, dtype, sharding=rep))
        else:
            arg_structs.append(jax.ShapeDtypeStruct(
                (shape[0] * NCORES,) + shape[1:], dtype, sharding=shd))
    for aval in out_avals:
        arg_structs.append(jax.ShapeDtypeStruct(
            (aval.shape[0] * NCORES,) + aval.shape[1:], aval.dtype,
            sharding=shd))

    def _compile_fn():
        f = jax.jit(
            shard_map(_body, mesh=mesh,
                      in_specs=in_specs,
                      out_specs=(PartitionSpec("core"),) * n_outs,
                      check_rep=False),
            donate_argnums=tuple(range(n_params, n_params + n_outs)),
            keep_unused=True)
        return f.lower(*arg_structs).compile()

    try:
        sharded = bass2jax.fast_dispatch_compile(_compile_fn)
    except Exception:
        sharded = jax.jit(
            shard_map(_body, mesh=mesh,
                      in_specs=in_specs,
                      out_specs=(PartitionSpec("core"),) * n_outs,
                      check_rep=False),
            donate_argnums=tuple(range(n_params, n_params + n_outs)),
            keep_unused=True)
    gather = jax.jit(lambda a: a, out_shardings=rep)   # upload-shard -> all-gather
    zeros = jax.jit(lambda: jnp.zeros((NCORES * T, D), jnp.bfloat16),
                    out_shardings=shd)
    reduce_y = jax.jit(lambda yc: yc.astype(jnp.float32).reshape(NCORES, T, D)
                       .sum(0), out_shardings=rep)
    _CACHE["runner"] = (sharded, in_names, out_names, out_avals,
                       mesh, rep, shd, gather, zeros, reduce_y)
    return _CACHE["runner"]


def _host_inputs(x, wq, wk, wv, wo):
    x = np.asarray(x, np.float32)
    wq = np.asarray(wq, np.float32)
    wk = np.asarray(wk, np.float32)
    wv = np.asarray(wv, np.float32)
    wo = np.asarray(wo, np.float32)

    xT = np.ascontiguousarray(x.reshape(T, D).T).astype(NPBF16)

    inv = 1.0 / (THETA ** (np.arange(0, HD, 2, dtype=np.float64) / HD))
    fr = np.outer(inv, np.arange(S, dtype=np.float64))   # [32, S]
    cosT = np.cos(fr).astype(NPBF16)
    sinT = np.sin(fr).astype(NPBF16)
    cos4 = np.ascontiguousarray(np.tile(cosT, (4, 1)))
    sin4 = np.ascontiguousarray(np.tile(sinT, (4, 1)))

    u = np.arange(896)[None, :]
    p = np.arange(128)[:, None]
    mask = (u >= p + 384).astype(NPBF16)

    in_maps = []
    for c in range(NCORES):
        cols_a, cols_b = [], []
        for h in range(HPC):
            base = (HPC * c + h) * HD
            cols_a.append(wq[:, base:base + 32])
            cols_b.append(wq[:, base + 32:base + 64])
        wq_c = np.concatenate(cols_a + cols_b, axis=1).astype(NPBF16)
        kb = c * HD
        wkv_c = np.concatenate(
            [wk[:, kb:kb + 32], wk[:, kb + 32:kb + 64], wv[:, kb:kb + HD]],
            axis=1).astype(NPBF16)
        wo_c = wo[c * HPC * HD:(c + 1) * HPC * HD, :].astype(NPBF16)
        in_maps.append({"xT": xT, "wq": wq_c, "wkv": wkv_c, "wo": wo_c,
                        "cos4": cos4, "sin4": sin4, "mask": mask})
    return in_maps


def _stage_inputs(in_maps):
    """Upload inputs: replicated tensors go up as 1/8 shards and are
    all-gathered on device; per-core tensors upload as the usual concat."""
    import jax
    (sharded, in_names, out_names, out_avals,
     mesh, rep, shd, gather, zeros, reduce_y) = _get_runner()
    staged = []
    for n in in_names:
        if n in REPLICATED:
            a = in_maps[0][n]
            if a.shape[0] % NCORES == 0:
                staged.append(gather(jax.device_put(a, shd)))
            else:
                staged.append(jax.device_put(a, rep))
        else:
            cat = np.concatenate([m[n] for m in in_maps], axis=0)
            staged.append(jax.device_put(cat, shd))
    return staged


def kernel(x, wq, wk, wv, wo):
    import jax
    (sharded, in_names, out_names, out_avals,
     mesh, rep, shd, gather, zeros, reduce_y) = _get_runner()
    in_maps = _host_inputs(x, wq, wk, wv, wo)
    staged = _stage_inputs(in_maps)
    out_arrs = sharded(*staged, zeros())
    ysum = reduce_y(out_arrs[out_names.index("y")])
    return np.asarray(ysum).reshape(B, S, D)


# revision 50
# speedup vs baseline: 183.3769x; 1.3888x over previous
"""Grouped-Query Attention (B=2, S=2048, D=2048, H=32, KV=8, HD=64) on 8 TRN2
NeuronCores, tensor-parallel over KV-head groups (1 KV head + 4 Q heads per
core), with host-side shard/gather.

v2: bf16 compute (halved HBM traffic, 2x DVE on 16-bit), pair-packed score
tiles with one exp per head-pair (Activation engine is the phase-2
bottleneck), PSUM retagged so scores double-buffer and the output projection
of tile jq overlaps attention of jq+1, engine rebalance (rope-K + copies on
Pool/Act, masks + normalize on DVE).

Per-core dataflow (activations feature-on-partitions; every matmul contracts
over the partition dim, no transposition of x):

  phase 1  QKV projection + RoPE, per 512-token tile
    psum: qa/qb (q ra/rb halves of 4 heads), kv = [ka|kb|v]; RoPE combines
    write q directly into qrot[128, hh, T] (head pairs in partition halves),
    k into krot rows 0:64 (replicated to 64:128 by per-tile DMA); V back to
    natural [tok, hd] via PE transpose.
  phase 2  attention per (batch, q-tile of 512), causal-block-skipped,
    head pairs processed in sequential i-loops:
      scoresT[2 x 128 kpos, 512 q] -> one exp per pair -> diag mask ->
      PV accumulate [65, 512] per head (row 64 = softmax denominator via
      ones column of V);  normalize -> a0/a1 (bf16);
    then the 512-token output projection y = [a0;a1].T @ wo on its own psum
    tags, overlapping the next q-tile's attention; y written bf16, host
    reduces the 8 row-sharded partials in f32.
"""

import contextlib
import numpy as np
import jax.numpy as jnp

import concourse.bass as bass
import concourse.tile as tile
from concourse import bacc, mybir
from concourse.masks import make_identity

B, S, D = 2, 2048, 2048
H, KV, HD = 32, 8, 64
T = B * S
NCORES = 8
HPC = H // NCORES          # 4 query heads per core
SCALE = 1.0 / np.sqrt(HD)
THETA = 10000.0
NQT = T // 512             # 8 token tiles of 512
REPLICATED = {"xT", "cos4", "sin4", "mask"}  # same bytes on every core
NDT = D // 128             # 16 contraction tiles
F32 = mybir.dt.float32
BF16 = mybir.dt.bfloat16
NPBF16 = mybir.dt.np(BF16)


def _build_program():
    nc = bacc.Bacc("TRN2", target_bir_lowering=False, debug=False)

    xT = nc.dram_tensor("xT", [D, T], BF16, kind="ExternalInput")
    wq = nc.dram_tensor("wq", [D, 2 * HPC * 32], BF16, kind="ExternalInput")
    wkv = nc.dram_tensor("wkv", [D, 128], BF16, kind="ExternalInput")
    wo = nc.dram_tensor("wo", [HPC * HD, D], BF16, kind="ExternalInput")
    cos4 = nc.dram_tensor("cos4", [128, S], BF16, kind="ExternalInput")
    sin4 = nc.dram_tensor("sin4", [128, S], BF16, kind="ExternalInput")
    maskd = nc.dram_tensor("mask", [128, 896], BF16, kind="ExternalInput")
    y = nc.dram_tensor("y", [T, D], BF16, kind="ExternalOutput")

    with tile.TileContext(nc) as tc:
        _body(tc, nc, xT, wq, wkv, wo, cos4, sin4, maskd, y)
    nc.compile()
    return nc


def _body(tc, nc, xT, wq, wkv, wo, cos4, sin4, maskd, y):
    TT = mybir.AluOpType
    EXP = mybir.ActivationFunctionType.Exp
    ctx = contextlib.ExitStack()
    with ctx:
        const = ctx.enter_context(tc.tile_pool(name="const", bufs=1))
        persist = ctx.enter_context(tc.tile_pool(name="persist", bufs=1))
        xs = ctx.enter_context(tc.tile_pool(name="xs", bufs=9))
        rtmp = ctx.enter_context(tc.tile_pool(name="rtmp", bufs=1))
        probs = ctx.enter_context(tc.tile_pool(name="probs", bufs=6))
        yout = ctx.enter_context(tc.tile_pool(name="yout", bufs=3))
        norm = ctx.enter_context(tc.tile_pool(name="norm", bufs=2))

        # PSUM: 16KB/partition, allocated exactly:
        #   T0..T3: [128,512] f32 (2KB = 1 bank each)
        #   S0,S1 : [128,2,512] f32 (4KB = 2 banks each)
        # phase 1: qa/qb even qt -> T0/T1, odd qt -> S0 halves; kv=T2, tp=T3
        # phase 2: scores alternate S0/S1; pv pair -> T2/T3; out-proj T0/T1
        psum = ctx.enter_context(tc.tile_pool(name="psum", bufs=1, space="PSUM"))

        def pT(i):
            return psum.tile([128, 512], F32, name=f"psT{i}")

        def pS(i):
            return psum.tile([128, 2, 512], F32, name=f"psS{i}")

        # ---- constants ----
        # load order = first-use order: wkv gates the very first matmul,
        # wq the first q matmuls, wo is not needed until phase 2.
        wkv_sb = const.tile([128, NDT, 128], BF16, name="wkv_sb")
        wq_sb = const.tile([128, NDT, 256], BF16, name="wq_sb")
        # quarter-wise loads: the first kv/q matmuls only need the first
        # d-tiles, so they start before the full weight DMA lands
        for q4 in range(4):
            dl = slice(q4 * NDT // 4, (q4 + 1) * NDT // 4)
            dr = slice(q4 * (D // 4), (q4 + 1) * (D // 4))
            nc.sync.dma_start(out=wkv_sb[:, dl, :],
                              in_=wkv[dr, :].rearrange("(t p) c -> p t c", p=128))
            nc.sync.dma_start(out=wq_sb[:, dl, :],
                              in_=wq[dr, :].rearrange("(t p) c -> p t c", p=128))
        # later-use constants are DMA'd after the first token tile's x loads
        # (the SP queue is in-order; see `if qt == 0` in the phase-1 loop)
        cos_sb = const.tile([128, S], BF16, name="cos_sb")
        sin_sb = const.tile([128, S], BF16, name="sin_sb")
        mask_sb = const.tile([128, 896], BF16, name="mask_sb")
        wo_sb = const.tile([128, 2, D], BF16, name="wo_sb")
        ident = const.tile([64, 64], F32, name="ident")
        make_identity(nc, ident)

        def load_late_consts():
            nc.sync.dma_start(out=cos_sb, in_=cos4[:, :])
            nc.sync.dma_start(out=sin_sb, in_=sin4[:, :])
            nc.sync.dma_start(out=mask_sb, in_=maskd[:, :])
            nc.sync.dma_start(out=wo_sb,
                              in_=wo[:, :].rearrange("(t p) c -> p t c", p=128))

        # ---- persistent activations ----
        # qrot: [64*pair + (0:32 ra | 32:64 rb), head-in-pair, tok]
        qrot = persist.tile([128, 2, T], BF16, name="qrot")
        krot = persist.tile([128, T], BF16, name="krot")   # 64:128 = replica
        vnat = persist.tile([128, T // 128, 65], BF16, name="vnat")
        a0 = persist.tile([128, T], BF16, name="a0")       # heads 0,1
        a1 = persist.tile([128, T], BF16, name="a1")       # heads 2,3
        ones_c = const.tile([128, T // 128, 1], F32, name="ones_c")
        nc.vector.memset(ones_c, 1.0)
        nc.vector.tensor_copy(out=vnat[:, :, 64:65], in_=ones_c)

        # ================= phase 1: projections + rope =================
        for qt in range(NQT):
            pos0 = (qt % 4) * 512
            tok0 = qt * 512
            if qt % 2 == 0:
                qa_ps, qb_ps = pT(0), pT(1)
            else:
                s_ps = pS(0)
                qa_ps, qb_ps = s_ps[:, 0, :], s_ps[:, 1, :]
            kv_ps = pT(2)
            # kv projection first: the single-buffered kv bank is consumed by
            # rope-K (DVE) while PE continues with the qa/qb matmuls below.
            # x tiles arrive 2 contraction blocks per DMA (fewer, larger
            # transfers -> half the HWDGE queue work).
            xts = []
            for d2 in range(NDT // 2):
                xt = xs.tile([128, 2, 512], BF16, name="xt")
                nc.sync.dma_start(
                    out=xt,
                    in_=xT[d2 * 256:(d2 + 1) * 256, tok0:tok0 + 512]
                    .rearrange("(g p) t -> p g t", p=128))
                xts.append(xt)
                for g in range(2):
                    d = 2 * d2 + g
                    nc.tensor.matmul(out=kv_ps, lhsT=wkv_sb[:, d, :],
                                     rhs=xt[:, g, :],
                                     start=d == 0, stop=d == NDT - 1)
            if qt == 0:
                load_late_consts()
            cs = cos_sb[:, pos0:pos0 + 512]
            sn = sin_sb[:, pos0:pos0 + 512]
            # stage k through SBUF bf16 (Act copy, idle engine) so every rope
            # TT runs in the DVE 16-bit fast mode
            kk = rtmp.tile([64, 512], BF16, name="kk")
            nc.scalar.copy(out=kk, in_=kv_ps[0:64])
            k_x = rtmp.tile([32, 512], BF16, name="k_x")
            k_x2 = rtmp.tile([32, 512], BF16, name="k_x2")
            k_y = rtmp.tile([32, 512], BF16, name="k_y")
            k_y2 = rtmp.tile([32, 512], BF16, name="k_y2")
            nc.vector.tensor_tensor(out=k_x, in0=kk[0:32], in1=cs[0:32], op=TT.mult)
            nc.vector.tensor_tensor(out=k_x2, in0=kk[0:32], in1=sn[0:32], op=TT.mult)
            # cos/sin tables are 4x-tiled over partitions, so rows 32:64
            # equal rows 0:32; index them to satisfy the equal-base-partition
            # rule for SBUF-SBUF TensorTensor.
            nc.vector.tensor_tensor(out=k_y, in0=kk[32:64], in1=sn[32:64], op=TT.mult)
            nc.vector.tensor_tensor(out=k_y2, in0=kk[32:64], in1=cs[32:64], op=TT.mult)
            vt = rtmp.tile([64, 512], F32, name="vt")
            nc.scalar.copy(out=vt, in_=kv_ps[64:128])
            nc.vector.tensor_tensor(out=krot[0:32, tok0:tok0 + 512], in0=k_x,
                                    in1=k_y, op=TT.subtract)
            nc.vector.tensor_tensor(out=krot[32:64, tok0:tok0 + 512], in0=k_x2,
                                    in1=k_y2, op=TT.add)
            nc.sync.dma_start(out=krot[64:128, tok0:tok0 + 512],
                              in_=krot[0:64, tok0:tok0 + 512])
            for d in range(NDT):
                st, sp = d == 0, d == NDT - 1
                xtg = xts[d // 2][:, d % 2, :]
                nc.tensor.matmul(out=qa_ps, lhsT=wq_sb[:, d, 0:128], rhs=xtg,
                                 start=st, stop=sp)
                nc.tensor.matmul(out=qb_ps, lhsT=wq_sb[:, d, 128:256], rhs=xtg,
                                 start=st, stop=sp)
            # V -> natural [tok, hd] via PE transpose into T3, Pool copy out
            tp = pT(3)
            for k4 in range(4):
                nc.tensor.transpose(tp[:, 64 * k4:64 * k4 + 64],
                                    vt[:, 128 * k4:128 * k4 + 128], ident)
            for k4 in range(4):
                nc.scalar.copy(out=vnat[:, qt * 4 + k4, 0:64],
                               in_=tp[:, 64 * k4:64 * k4 + 64])
            # Q rope on DVE: [128, 512] (row 32h+r = head h, ra/rb dim r);
            # staged to SBUF bf16 by Act copies for the DVE fast mode
            qa_sb = rtmp.tile([128, 512], BF16, name="qa_sb")
            qb_sb = rtmp.tile([128, 512], BF16, name="qb_sb")
            nc.scalar.copy(out=qa_sb, in_=qa_ps)
            nc.scalar.copy(out=qb_sb, in_=qb_ps)
            t_x = rtmp.tile([128, 512], BF16, name="t_x")
            t_x2 = rtmp.tile([128, 512], BF16, name="t_x2")
            t_y = rtmp.tile([128, 512], BF16, name="t_y")
            t_y2 = rtmp.tile([128, 512], BF16, name="t_y2")
            nc.vector.tensor_tensor(out=t_x, in0=qa_sb, in1=cs, op=TT.mult)
            nc.vector.tensor_tensor(out=t_x2, in0=qa_sb, in1=sn, op=TT.mult)
            nc.vector.tensor_tensor(out=t_y, in0=qb_sb, in1=sn, op=TT.mult)
            nc.vector.tensor_tensor(out=t_y2, in0=qb_sb, in1=cs, op=TT.mult)
            for h in range(HPC):
                p, hh = h // 2, h % 2
                r0 = 32 * h
                nc.vector.tensor_tensor(
                    out=qrot[64 * p:64 * p + 32, hh, tok0:tok0 + 512],
                    in0=t_x[r0:r0 + 32], in1=t_y[r0:r0 + 32], op=TT.subtract)
                nc.vector.tensor_tensor(
                    out=qrot[64 * p + 32:64 * p + 64, hh, tok0:tok0 + 512],
                    in0=t_x2[r0:r0 + 32], in1=t_y2[r0:r0 + 32], op=TT.add)

        # ================= phase 2: attention + out-proj =================
        # Out-projection of q-tile jq runs on its own psum tags (T0/T1),
        # drained one 2-matmul unit per attention iteration of the NEXT
        # q-tile so the in-order PE queue never stalls the exp stream.
        def emit_outproj_unit(ts, n, par):
            yo = pT(par)
            nc.tensor.matmul(out=yo, lhsT=a0[:, ts:ts + 128],
                             rhs=wo_sb[:, 0, 512 * n:512 * n + 512],
                             start=True, stop=False)
            nc.tensor.matmul(out=yo, lhsT=a1[:, ts:ts + 128],
                             rhs=wo_sb[:, 1, 512 * n:512 * n + 512],
                             start=False, stop=True)
            ys = yout.tile([128, 512], BF16, name="ys")
            nc.vector.tensor_copy(out=ys, in_=yo)
            nc.sync.dma_start(out=y[ts:ts + 128, 512 * n:512 * n + 512], in_=ys)

        pending = []
        for b in range(B):
            for jq in range(4):
                tq = b * S + jq * 512
                ni = 4 * jq + 4
                for p in range(2):  # head pair: heads 2p, 2p+1
                    pv = [pT(2), pT(3)]
                    # 2-ahead software pipeline: PE queue per i is
                    # [scores(i), pv(i-2), outproj-unit] so exp(i-1) is never
                    # behind a matmul that waits on it.
                    # Diagonal blocks (offset r = i-4jq >= 0): q-chunks below
                    # the diagonal are fully masked -- skip them by computing
                    # only the q-span [128r, 512). The remaining triangular
                    # 128x128 chunk sits at span offset 0 and always uses the
                    # same mask slice (mask[p, 384+c] = c >= p).
                    def emit_pv(j):
                        ptj, q0, w = pts.pop(j)
                        for hh in range(2):
                            nc.tensor.matmul(out=pv[hh][0:65, q0:512],
                                             lhsT=vnat[:, b * 16 + j, :],
                                             rhs=ptj[:, hh, 0:w],
                                             start=(j == 0), stop=(j == ni - 1),
                                             skip_group_check=True)

                    pts = {}
                    for i in range(ni):
                        tk = b * S + i * 128
                        r = i - 4 * jq
                        q0 = 128 * r if r > 0 else 0
                        w = 512 - q0
                        sc = pS(i % 2)
                        for hh in range(2):
                            nc.tensor.matmul(
                                out=sc[:, hh, 0:w],
                                lhsT=krot[64 * p:64 * p + 64, tk:tk + 128],
                                rhs=qrot[64 * p:64 * p + 64, hh,
                                         tq + q0:tq + 512],
                                start=True, stop=True)
                        pt = probs.tile([128, 2, 512], BF16, name="pt")
                        nc.scalar.activation(out=pt[:, :, 0:w], in_=sc[:, :, 0:w],
                                             func=EXP, scale=float(SCALE))
                        if r >= 0:  # triangular chunk of the diagonal block
                            for hh in range(2):
                                nc.vector.tensor_tensor(
                                    out=pt[:, hh, 0:128], in0=pt[:, hh, 0:128],
                                    in1=mask_sb[:, 384:512], op=TT.mult)
                        pts[i] = (pt, q0, w)
                        if i >= 3:
                            emit_pv(i - 3)
                        # drain from slot 2 on: the first unit reads a0/a1
                        # written by a normalize that is still in flight at
                        # slot 0, and would block the in-order PE queue.
                        if pending and (p > 0 or i >= 2):
                            emit_outproj_unit(*pending.pop(0))
                    for j in (ni - 3, ni - 2, ni - 1):
                        emit_pv(j)
                    # normalize: row 64 of pv[hh] is the softmax denominator
                    sums = norm.tile([1, 1024], F32, name="sums")
                    for hh in range(2):
                        nc.scalar.copy(out=sums[0:1, 512 * hh:512 * hh + 512],
                                       in_=pv[hh][64:65, :])
                    rec = norm.tile([1, 1024], F32, name="rec")
                    nc.vector.reciprocal(out=rec, in_=sums)
                    rbc = norm.tile([64, 2, 512], F32, name="rbc")
                    for hh in range(2):
                        nc.gpsimd.partition_broadcast(
                            rbc[:, hh, :], rec[0:1, 512 * hh:512 * hh + 512])
                    dst = a0 if p == 0 else a1
                    for hh in range(2):
                        nc.vector.tensor_tensor(
                            out=dst[64 * hh:64 * hh + 64, tq:tq + 512],
                            in0=pv[hh][0:64, :], in1=rbc[:, hh, :], op=TT.mult)
                while pending:
                    emit_outproj_unit(*pending.pop(0))
                pending = [(tq + 128 * ci, n, (4 * ci + n) % 2)
                           for ci in range(4) for n in range(4)]
        while pending:
            emit_outproj_unit(*pending.pop(0))


_CACHE = {}


def _get_program():
    if "nc" not in _CACHE:
        _CACHE["nc"] = _build_program()
    return _CACHE["nc"]


def _get_runner():
    """Cached jitted shard_map executable over 8 cores (avoids per-call
    retrace that run_bass_kernel_spmd pays)."""
    if "runner" in _CACHE:
        return _CACHE["runner"]
    import jax
    from jax.sharding import Mesh, PartitionSpec
    from jax.experimental.shard_map import shard_map
    from concourse import bass2jax
    from concourse.bass2jax import _bass_exec_p

    bass2jax.install_neuronx_cc_hook()
    nc = _get_program()
    partition_name = nc.partition_id_tensor.name if nc.partition_id_tensor else None
    in_names, out_names, out_avals, in_shapes = [], [], [], {}
    for alloc in nc.m.functions[0].allocations:
        if not isinstance(alloc, mybir.MemoryLocationSet):
            continue
        name = alloc.memorylocations[0].name
        if alloc.kind == "ExternalInput":
            if name != partition_name:
                in_names.append(name)
                in_shapes[name] = (tuple(alloc.tensor_shape),
                                   mybir.dt.np(alloc.dtype))
        elif alloc.kind == "ExternalOutput":
            out_names.append(name)
            out_avals.append(jax.core.ShapedArray(
                tuple(alloc.tensor_shape), mybir.dt.np(alloc.dtype)))
    n_params = len(in_names)
    n_outs = len(out_avals)
    all_in = list(in_names) + list(out_names)
    if partition_name is not None:
        all_in.append(partition_name)

    def _body(*args):
        operands = list(args)
        if partition_name is not None:
            operands.append(bass2jax.partition_id_tensor())
        return tuple(_bass_exec_p.bind(
            *operands,
            out_avals=tuple(out_avals),
            in_names=tuple(all_in),
            out_names=tuple(out_names),
            lowering_input_output_aliases=(),
            sim_require_finite=True,
            sim_require_nnan=True,
            nc=nc,
        ))

    devices = jax.devices()[:NCORES]
    mesh = Mesh(np.asarray(devices), ("core",))
    # xT / rope tables / mask are identical on every core: feed them
    # replicated (P()) so the host uploads one copy + on-device all-gather,
    # instead of 8 copies through the tunnel.
    in_specs = tuple(
        PartitionSpec() if n in REPLICATED else PartitionSpec("core")
        for n in in_names) + (PartitionSpec("core"),) * n_outs

    from jax.sharding import NamedSharding
    rep = NamedSharding(mesh, PartitionSpec())
    shd = NamedSharding(mesh, PartitionSpec("core"))

    # AOT-compile with the bass effect suppressed (C++ fast-path dispatch):
    # per-call Python jit dispatch through the tunnel costs ~0.5ms, several
    # times the kernel's own execution time.
    arg_structs = []
    for n in in_names:
        shape, dtype = in_shapes[n]
        if n in REPLICATED:
            arg_structs.append(jax.ShapeDtypeStruct(shape, dtype, sharding=rep))
        else:
            arg_structs.append(jax.ShapeDtypeStruct(
                (shape[0] * NCORES,) + shape[1:], dtype, sharding=shd))
    for aval in out_avals:
        arg_structs.append(jax.ShapeDtypeStruct(
            (aval.shape[0] * NCORES,) + aval.shape[1:], aval.dtype,
            sharding=shd))

    def _compile_fn():
        f = jax.jit(
            shard_map(_body, mesh=mesh,
                      in_specs=in_specs,
                      out_specs=(PartitionSpec("core"),) * n_outs,
                      check_rep=False),
            donate_argnums=tuple(range(n_params, n_params + n_outs)),
            keep_unused=True)
        return f.lower(*arg_structs).compile()

    try:
        sharded = bass2jax.fast_dispatch_compile(_compile_fn)
    except Exception:
        sharded = jax.jit(
            shard_map(_body, mesh=mesh,
                      in_specs=in_specs,
                      out_specs=(PartitionSpec("core"),) * n_outs,
                      check_rep=False),
            donate_argnums=tuple(range(n_params, n_params + n_outs)),
            keep_unused=True)
    gather = jax.jit(lambda a: a, out_shardings=rep)   # upload-shard -> all-gather
    zeros = jax.jit(lambda: jnp.zeros((NCORES * T, D), jnp.bfloat16),
                    out_shardings=shd)
    reduce_y = jax.jit(lambda yc: yc.astype(jnp.float32).reshape(NCORES, T, D)
                       .sum(0), out_shardings=rep)
    _CACHE["runner"] = (sharded, in_names, out_names, out_avals,
                       mesh, rep, shd, gather, zeros, reduce_y)
    return _CACHE["runner"]


def _host_inputs(x, wq, wk, wv, wo):
    x = np.asarray(x, np.float32)
    wq = np.asarray(wq, np.float32)
    wk = np.asarray(wk, np.float32)
    wv = np.asarray(wv, np.float32)
    wo = np.asarray(wo, np.float32)

    xT = np.ascontiguousarray(x.reshape(T, D).T).astype(NPBF16)

    inv = 1.0 / (THETA ** (np.arange(0, HD, 2, dtype=np.float64) / HD))
    fr = np.outer(inv, np.arange(S, dtype=np.float64))   # [32, S]
    cosT = np.cos(fr).astype(NPBF16)
    sinT = np.sin(fr).astype(NPBF16)
    cos4 = np.ascontiguousarray(np.tile(cosT, (4, 1)))
    sin4 = np.ascontiguousarray(np.tile(sinT, (4, 1)))

    u = np.arange(896)[None, :]
    p = np.arange(128)[:, None]
    mask = (u >= p + 384).astype(NPBF16)

    in_maps = []
    for c in range(NCORES):
        cols_a, cols_b = [], []
        for h in range(HPC):
            base = (HPC * c + h) * HD
            cols_a.append(wq[:, base:base + 32])
            cols_b.append(wq[:, base + 32:base + 64])
        wq_c = np.concatenate(cols_a + cols_b, axis=1).astype(NPBF16)
        kb = c * HD
        wkv_c = np.concatenate(
            [wk[:, kb:kb + 32], wk[:, kb + 32:kb + 64], wv[:, kb:kb + HD]],
            axis=1).astype(NPBF16)
        wo_c = wo[c * HPC * HD:(c + 1) * HPC * HD, :].astype(NPBF16)
        in_maps.append({"xT": xT, "wq": wq_c, "wkv": wkv_c, "wo": wo_c,
                        "cos4": cos4, "sin4": sin4, "mask": mask})
    return in_maps


def _stage_inputs(in_maps):
    """Upload inputs: replicated tensors go up as 1/8 shards and are
    all-gathered on device; per-core tensors upload as the usual concat."""
    import jax
    (sharded, in_names, out_names, out_avals,
     mesh, rep, shd, gather, zeros, reduce_y) = _get_runner()
    staged = []
    for n in in_names:
        if n in REPLICATED:
            a = in_maps[0][n]
            if a.shape[0] % NCORES == 0:
                staged.append(gather(jax.device_put(a, shd)))
            else:
                staged.append(jax.device_put(a, rep))
        else:
            cat = np.concatenate([m[n] for m in in_maps], axis=0)
            staged.append(jax.device_put(cat, shd))
    return staged


def kernel(x, wq, wk, wv, wo):
    import jax
    (sharded, in_names, out_names, out_avals,
     mesh, rep, shd, gather, zeros, reduce_y) = _get_runner()
    in_maps = _host_inputs(x, wq, wk, wv, wo)
    staged = _stage_inputs(in_maps)
    out_arrs = sharded(*staged, zeros())
    ysum = reduce_y(out_arrs[out_names.index("y")])
    return np.asarray(ysum).reshape(B, S, D)
